# revision 1
# baseline (speedup 1.0000x reference)
"""YOLOv5 detection-loss (DetLoss) Trainium2 Bass kernel, 8-core SPMD.

Strategy
--------
The loss decomposes so that the only dense work over the big prediction
tensors p0/p1/p2 is a softplus-sum over channel 4 (the objectness logit):

    mean(BCE(x, tobj)) = [ sum_grid softplus(x) - sum_pos tobj_cell * x_cell ] / G

(BCE(x,t) - BCE(x,0) = -t*x, and BCE(x,0) = softplus(x)).  Likewise the
class loss reduces to sum softplus(pcls) - sum pcls[row, tcls-1] over the
gathered positive rows.  The box (CIoU) loss only needs the gathered
positive rows.  Everything else (one-hot selection, scatter-duplicate
resolution, denominators, the final weighted combine) is index
arithmetic done host-side.

Sharding: data-parallel over batch; core k owns images [2k, 2k+2) of
every layer and the positive rows whose image id falls in that range.
Each core writes a [128, OUTC] tile of partial sums; the host reduces
over partitions and cores and applies the weights.
"""

import os
import numpy as np

# ---------------- problem constants (YOLOv5s / COCO head) ----------------
B, NA, NCLS, NO = 16, 3, 80, 85
NL = 3
NCORES = 8
BPC = B // NCORES  # images per core
BALANCE = (4.0, 1.0, 0.4)
HYP_BOX, HYP_OBJ, HYP_CLS = 0.05, 1.0, 0.05
EPS = 1e-7
P = 128  # SBUF partitions
# "strided" loads only the objectness channel (one 4B descriptor per grid
# cell, ~22us of descriptor time) instead of streaming full rows (~48us at
# contiguous bandwidth): HW-measured 47.6us vs 62.4us end-to-end, rel err
# 1.3e-3 vs 9.4e-4, both comfortably inside tolerance.
GRID_MODE = os.environ.get("DETLOSS_GRID_MODE", "strided")  # "strided" | "contig"
MAX_CHUNK_M = 75  # grid-chunk free size in units of 85-float cells per partition

_cache: dict = {}


def _grid_chunks(cells: int):
    """Split a layer shard of `cells` grid cells into [(P_l, m0, m1), ...]
    chunks where the SBUF tile is [P_l, (m1-m0)*85] and every partition
    starts at a cell boundary (so channel 4 sits at free offset 4+85*j)."""
    pl = P
    while cells % pl:
        pl -= 1
    m_tot = cells // pl
    out = []
    j = 0
    while j < m_tot:
        m = min(MAX_CHUNK_M, m_tot - j)
        out.append((pl, j, j + m))
        j += m
    return out


def _build_program(layer_shapes, Ts, grid_mode):
    """Build the SPMD Bass program.

    layer_shapes: [(gh, gw)] * 3;  Ts: padded slot-columns per layer.
    Returns (nc, meta) where meta has the accumulator column map.
    """
    import concourse.bass as bass
    import concourse.mybir as mybir
    import concourse.tile as tile

    f32 = mybir.dt.float32
    i32 = mybir.dt.int32
    ALU = mybir.AluOpType
    ACTF = mybir.ActivationFunctionType
    COLS = sum(Ts)
    c_offs = np.concatenate([[0], np.cumsum(Ts)]).astype(int)  # layer col ranges

    nc = bass.Bass()

    p_handles = []
    rows_l = []
    for l, (gh, gw) in enumerate(layer_shapes):
        rows = BPC * NA * gh * gw
        rows_l.append(rows)
        p_handles.append(
            nc.declare_dram_parameter(f"p{l}s", [rows, NO], f32, isOutput=False)
        )
    POSR = nc.declare_dram_parameter("posrows", [P, COLS * NO], f32, isOutput=False)
    OHV = nc.declare_dram_parameter("ohvals", [P, COLS], f32, isOutput=False)
    # consts layout per partition: 4 paired blocks [COLS,2] (awh2, b2min,
    # b2max, cxy2) then 4 single blocks [COLS] (w2h2pe, atan2c, wbox, wdedup)
    NCONST = 12
    CST = nc.declare_dram_parameter("consts", [P, NCONST * COLS], f32, isOutput=False)

    # accumulator column map
    col_box = [0, 1, 2]
    col_corr = [3, 4, 5]
    col_clssp = [6, 7, 8]
    col_oh = [9, 10, 11]
    grid_cols = {}  # layer -> list of acc cols
    ncol = 12
    chunks = []
    for l in range(NL):
        gh, gw = layer_shapes[l]
        cells = BPC * NA * gh * gw
        # both modes use <=75-cell accumulation chunks; strided mode just
        # loads each layer's channel-4 plane with one strided DMA instead
        # of streaming the full rows
        ch = _grid_chunks(cells)
        chunks.append(ch)
        grid_cols[l] = list(range(ncol, ncol + len(ch)))
        ncol += len(ch)
    OUTC = ncol
    OUT = nc.declare_dram_parameter("partial", [P, OUTC], f32, isOutput=True)

    with tile.TileContext(nc) as tc:
        with (
            tc.tile_pool(name="grid", bufs=3) as gpool,
            tc.tile_pool(name="sp", bufs=2) as sppool,
            tc.tile_pool(name="small", bufs=1) as sm,
        ):
            # ---------- small input loads ----------
            cst = sm.tile([P, NCONST * COLS], f32)
            nc.sync.dma_start(out=cst[:], in_=CST[:])

            acc = sm.tile([P, OUTC], f32)
            nc.vector.memset(acc[:], 0.0)

            # const views
            def paired(o):  # -> [P, COLS, 2]
                return cst[:, o * 2 * COLS : (o + 1) * 2 * COLS].rearrange(
                    "p (t c) -> p t c", c=2
                )

            def single(o):  # -> [P, COLS]
                return cst[:, 8 * COLS + o * COLS : 8 * COLS + (o + 1) * COLS]

            awh2, b2min, b2max, cxy2 = paired(0), paired(1), paired(2), paired(3)
            w2h2pe, atan2c, wbox, wdedup = (
                single(0),
                single(1),
                single(2),
                single(3),
            )

            # ---------- positive rows (host-gathered input) ----------
            pos = sm.tile([P, COLS * NO], f32)
            ohg = sm.tile([P, COLS], f32)
            nc.sync.dma_start(out=pos[:], in_=POSR[:])
            nc.sync.dma_start(out=ohg[:], in_=OHV[:])
            pos_v = pos[:].rearrange("p (t c) -> p t c", c=NO)

            # ---------- phase 1 on ACT: sigmoid + arctan (one table set) ----
            # per-layer ops where the input is gather-produced: each indirect
            # DMA completes on its own sem lane and the ACT/DVE ISA encodings
            # only carry 2 sync waits, so never read all three layers' pos in
            # one instruction.
            sig = sm.tile([P, COLS * 4], f32)
            sig_v = sig[:].rearrange("p (t c) -> p t c", c=4)
            for l in range(NL):
                c0, c1 = int(c_offs[l]), int(c_offs[l + 1])
                i_sig = nc.scalar.activation(
                    out=sig_v[:, c0:c1, :], in_=pos_v[:, c0:c1, 0:4], func=ACTF.Sigmoid
                )

            _tn = [0]

            def pair_tile():
                _tn[0] += 1
                return sm.tile([P, COLS * 2], f32, name=f"pair{_tn[0]}")

            def pv(t):  # view [P, COLS, 2]
                return t[:].rearrange("p (t c) -> p t c", c=2)

            V = nc.vector
            pxy, swh2, pwhh = pair_tile(), pair_tile(), pair_tile()
            b1min, b1max = pair_tile(), pair_tile()
            tmpa, tmpb = pair_tile(), pair_tile()

            V.tensor_scalar(pv(pxy)[:], sig_v[:, :, 0:2], 2.0, -0.5, ALU.mult, ALU.add)
            V.tensor_tensor(pv(swh2)[:], sig_v[:, :, 2:4], sig_v[:, :, 2:4], ALU.mult)
            V.tensor_tensor(pv(pwhh)[:], pv(swh2)[:], awh2[:], ALU.mult)
            V.tensor_tensor(pv(b1min)[:], pv(pxy)[:], pv(pwhh)[:], ALU.subtract)
            V.tensor_tensor(pv(b1max)[:], pv(pxy)[:], pv(pwhh)[:], ALU.add)

            def stile():
                _tn[0] += 1
                return sm.tile([P, COLS], f32, name=f"s{_tn[0]}")

            # intersection
            V.tensor_tensor(pv(tmpa)[:], pv(b1max)[:], b2max[:], ALU.min)  # imin
            V.tensor_tensor(pv(tmpb)[:], pv(b1min)[:], b2min[:], ALU.max)  # imax
            V.tensor_tensor(pv(tmpa)[:], pv(tmpa)[:], pv(tmpb)[:], ALU.subtract)
            V.tensor_scalar(pv(tmpa)[:], pv(tmpa)[:], 0.0, None, ALU.max)  # relu
            inter = stile()
            V.tensor_tensor(inter[:], pv(tmpa)[:, :, 0], pv(tmpa)[:, :, 1], ALU.mult)
            # union (w1h1 = 4 * pwhh_x * pwhh_y)
            area4, u, ru, iou = stile(), stile(), stile(), stile()
            V.tensor_tensor(area4[:], pv(pwhh)[:, :, 0], pv(pwhh)[:, :, 1], ALU.mult)
            V.tensor_scalar(u[:], area4[:], 4.0, None, ALU.mult)
            V.tensor_tensor(u[:], u[:], w2h2pe[:], ALU.add)
            V.tensor_tensor(u[:], u[:], inter[:], ALU.subtract)
            V.reciprocal(ru[:], u[:])
            V.tensor_tensor(iou[:], inter[:], ru[:], ALU.mult)
            # enclosing box diag^2
            V.tensor_tensor(pv(tmpa)[:], pv(b1max)[:], b2max[:], ALU.max)
            V.tensor_tensor(pv(tmpb)[:], pv(b1min)[:], b2min[:], ALU.min)
            V.tensor_tensor(pv(tmpa)[:], pv(tmpa)[:], pv(tmpb)[:], ALU.subtract)
            V.tensor_tensor(pv(tmpa)[:], pv(tmpa)[:], pv(tmpa)[:], ALU.mult)
            c2 = stile()
            V.tensor_tensor(c2[:], pv(tmpa)[:, :, 0], pv(tmpa)[:, :, 1], ALU.add)
            V.tensor_scalar(c2[:], c2[:], EPS, None, ALU.add)
            # center distance^2
            V.tensor_tensor(pv(tmpb)[:], pv(pxy)[:], cxy2[:], ALU.subtract)
            V.tensor_tensor(pv(tmpb)[:], pv(tmpb)[:], pv(tmpb)[:], ALU.mult)
            rho2, rc2, rr = stile(), stile(), stile()
            V.tensor_tensor(rho2[:], pv(tmpb)[:, :, 0], pv(tmpb)[:, :, 1], ALU.add)
            V.reciprocal(rc2[:], c2[:])
            V.tensor_tensor(rr[:], rho2[:], rc2[:], ALU.mult)
            # v-term: atan(w2/(h2+eps)) - atan(w1/(h1+eps));
            # w1/(h1+eps) == pwhh_x/(pwhh_y+eps/2)
            denh, q = stile(), stile()
            V.tensor_scalar(denh[:], pv(pwhh)[:, :, 1], EPS * 0.5, None, ALU.add)
            V.reciprocal(denh[:], denh[:])
            V.tensor_tensor(q[:], pv(pwhh)[:, :, 0], denh[:], ALU.mult)
            # ACT Arctan only supports [-pi/2, pi/2]; q > 0, so use
            # atan(q) = pi/2 - atan(1/q) for q > 1 (branchless select).
            rq, qm, at, mgt, u2 = stile(), stile(), stile(), stile(), stile()
            V.reciprocal(rq[:], q[:])
            V.tensor_tensor(qm[:], q[:], rq[:], ALU.min)
            i_at = nc.scalar.activation(out=at[:], in_=qm[:], func=ACTF.Arctan)
            V.tensor_scalar(mgt[:], q[:], 1.0, None, ALU.is_gt)
            V.tensor_scalar(u2[:], at[:], -2.0, float(np.pi / 2), ALU.mult, ALU.add)
            V.tensor_tensor(u2[:], mgt[:], u2[:], ALU.mult)
            V.tensor_tensor(at[:], at[:], u2[:], ALU.add)
            dat, v4 = stile(), stile()
            V.tensor_tensor(dat[:], atan2c[:], at[:], ALU.subtract)
            V.tensor_tensor(v4[:], dat[:], dat[:], ALU.mult)
            V.tensor_scalar(v4[:], v4[:], float(4.0 / np.pi**2), None, ALU.mult)
            ad, rad, alpha, va = stile(), stile(), stile(), stile()
            V.tensor_tensor(ad[:], v4[:], iou[:], ALU.subtract)
            V.tensor_scalar(ad[:], ad[:], 1.0 + EPS, None, ALU.add)
            V.reciprocal(rad[:], ad[:])
            V.tensor_tensor(alpha[:], v4[:], rad[:], ALU.mult)
            V.tensor_tensor(va[:], v4[:], alpha[:], ALU.mult)
            ciou = stile()
            V.tensor_tensor(ciou[:], iou[:], rr[:], ALU.subtract)
            V.tensor_tensor(ciou[:], ciou[:], va[:], ALU.subtract)

            # per-layer reductions from the ciou tile
            omc, rel, rp4 = stile(), stile(), stile()
            V.tensor_scalar(omc[:], ciou[:], -1.0, 1.0, ALU.mult, ALU.add)
            V.tensor_scalar(rel[:], ciou[:], 0.0, None, ALU.max)
            for l in range(NL):
                c0, c1 = int(c_offs[l]), int(c_offs[l + 1])
                V.tensor_tensor(
                    rp4[:, c0:c1], rel[:, c0:c1], pos_v[:, c0:c1, 4], ALU.mult
                )
            # masked per-layer sums (tensor_tensor_reduce is an extended ISA
            # op this compiler can't encode, so use mult + reduce pairs)
            X = mybir.AxisListType.X

            def masked_sum(src, mask, acc_col, w):
                s = stile()
                V.tensor_tensor(s[:, :w], src, mask, ALU.mult)
                V.reduce_sum(acc[:, acc_col : acc_col + 1], s[:, :w], X)

            for l in range(NL):
                c0, c1 = int(c_offs[l]), int(c_offs[l + 1])
                w = c1 - c0
                masked_sum(omc[:, c0:c1], wbox[:, c0:c1], col_box[l], w)
                masked_sum(rp4[:, c0:c1], wdedup[:, c0:c1], col_corr[l], w)
                masked_sum(ohg[:, c0:c1], wbox[:, c0:c1], col_oh[l], w)

            # ---------- phase 2 on ACT: softplus = ln(1 + exp(x)) ----------
            # (this compiler's softplus table set lacks the softplus entry,
            # so synthesize it; exp and ln share natural_log_exp_and_others)
            sp_acts = []
            for l in range(NL):
                c0, c1 = int(c_offs[l]), int(c_offs[l + 1])
                tl = Ts[l]
                spe = sppool.tile([P, tl * NCLS], f32, tag="spclse")
                spc = sppool.tile([P, tl * NCLS], f32, tag="spcls")
                i1 = nc.scalar.activation(
                    out=spe[:].rearrange("p (t c) -> p t c", c=NCLS),
                    in_=pos_v[:, c0:c1, 5:NO],
                    func=ACTF.Exp,
                )
                i = nc.scalar.activation(
                    out=spc[:], in_=spe[:], func=ACTF.Ln, bias=1.0
                )
                sp_acts.append(i1)
                sp_acts.append(i)
                spr = sm.tile([P, tl], f32, tag=f"spr{l}")
                V.reduce_sum(
                    spr[:],
                    spc[:].rearrange("p (t c) -> p t c", c=NCLS),
                    mybir.AxisListType.X,
                )
                masked_sum(spr[:], wbox[:, c0:c1], col_clssp[l], tl)

            # grid softplus
            for l in range(NL):
                gv = p_handles[l][:].rearrange(
                    "(p x) c -> p x c", p=chunks[l][0][0]
                )
                gt_layer = None
                if grid_mode != "contig":
                    pl0 = chunks[l][0][0]
                    m_tot = chunks[l][-1][2]
                    gt_layer = gpool.tile([P, m_tot], f32, name=f"gts{l}")
                    nc.sync.dma_start(out=gt_layer[:pl0, :], in_=gv[:, :, 4])
                for j, (pl, m0, m1) in enumerate(chunks[l]):
                    m = m1 - m0
                    if grid_mode == "contig":
                        gt = gpool.tile([P, MAX_CHUNK_M * NO], f32, tag="grid")
                        nc.sync.dma_start(
                            out=gt[:pl, : m * NO].rearrange("p (x c) -> p x c", c=NO),
                            in_=gv[:, m0:m1, :],
                        )
                        src = gt[:pl, 4 : m * NO : NO]
                    else:
                        src = gt_layer[:pl, m0:m1]
                    spe = sppool.tile([P, m], f32, tag="spgride")
                    spg = sppool.tile([P, m], f32, tag="spgrid")
                    col = grid_cols[l][j]
                    i1 = nc.scalar.activation(out=spe[:pl, :m], in_=src, func=ACTF.Exp)
                    i = nc.scalar.activation(
                        out=spg[:pl, :m],
                        in_=spe[:pl, :m],
                        func=ACTF.Ln,
                        bias=1.0,
                        accum_out=acc[:pl, col : col + 1],
                    )
                    sp_acts.append(i1)
                    sp_acts.append(i)

            # enforce ACT ordering: all softplus after arctan (2 table loads
            # total: sigmoid_and_others once, softplus_and_others once)
            for i in sp_acts:
                tile.add_dep_helper(i.ins, i_at.ins, False, "act-table-grouping")

            # ---------- store partials ----------
            # joiner copy: the store would otherwise wait on ACT + DVE + a DMA
            # lane (3 sems); the copy folds ACT/DVE into one DVE tick.
            acc2 = sm.tile([P, OUTC], f32)
            V.tensor_copy(acc2[:], acc[:])
            nc.sync.dma_start(out=OUT[:], in_=acc2[:])

    _cap_sync_waits(nc, mybir)
    nc.finalize()
    meta = dict(
        COLS=COLS,
        c_offs=c_offs,
        OUTC=OUTC,
        col_box=col_box,
        col_corr=col_corr,
        col_clssp=col_clssp,
        col_oh=col_oh,
        grid_cols=grid_cols,
        NCONST=NCONST,
    )
    return nc, meta


def _cap_sync_waits(nc, mybir, maxw=1):
    """Compute-engine ISA encodings carry very few sync waits; Tile's
    scheduler can emit more (one per DMA sem lane).  Two rewrites, both
    semantics-preserving:
      1. drop waits on the instruction's own engine-completion semaphore
         (engine program order already guarantees them);
      2. hoist waits beyond `maxw` onto standalone EventSemaphore
         instructions placed just before the offender on the same engine.
    """
    # engine -> its completion-sem name prefix (as assigned by Tile)
    eng_sem = {
        "DVE": "DVE",
        "Activation": "Activation",
        "SP": "SP",
        "Pool": "Pool",
        "PE": "PE",
    }
    # sem id -> ant_name, for the RANGE_CLEAR rewrite below
    sem_names = {}
    for bb in nc.m.functions[0].blocks:
        for inst in bb.instructions:
            si = getattr(inst, "sync_info", None)
            if not si:
                continue
            for w in si.on_wait or []:
                sem_names[w.id] = w.ant_name
            for u in si.on_update or []:
                sem_names[u.id] = u.ant_name

    rc_opcode = 176  # NEURON_ISA_TPB_OPCODE_EVENT_SEMAPHORE_RANGE_CLEAR
    n = 0
    for bb in nc.m.functions[0].blocks:
        out = []
        for inst in bb.instructions:
            tname = type(inst).__name__
            if tname == "InstISA" and getattr(inst, "isa_opcode", None) == rc_opcode:
                # this walrus build can't codegen RANGE_CLEAR; emit one
                # sem-wr-imm 0 EventSemaphore per sem in the range instead
                start, end = inst.instr[13], inst.instr[14]
                for sid in range(start, end + 1):
                    out.append(
                        mybir.InstEventSemaphore(
                            name=f"W-semreset-{sid}",
                            engine=inst.engine,
                            sync_info=mybir.SyncInfo(
                                on_wait=[],
                                on_update=[
                                    mybir.SyncUpdate(
                                        sync_type="semaphore",
                                        id=sid,
                                        update_mode="sem-wr-imm",
                                        update_value=0,
                                        ant_name=sem_names.get(sid, f"sem{sid}"),
                                    )
                                ],
                            ),
                        )
                    )
                continue
            si = getattr(inst, "sync_info", None)
            ow = list(si.on_wait) if (si and si.on_wait) else []
            if ow and tname != "InstEventSemaphore":
                epfx = eng_sem.get(str(inst.engine).split(".")[-1])
                if epfx:
                    keep0 = [
                        w
                        for w in ow
                        if not (w.ant_name or "").startswith(epfx + "_")
                    ]
                else:
                    keep0 = ow
                if len(keep0) > maxw:
                    excess, keep = keep0[:-maxw], keep0[-maxw:]
                    for w in excess:
                        n += 1
                        out.append(
                            mybir.InstEventSemaphore(
                                name=f"W-cap-{n}",
                                engine=inst.engine,
                                sync_info=mybir.SyncInfo(on_wait=[w], on_update=[]),
                            )
                        )
                else:
                    keep = keep0
                if len(keep) != len(ow):
                    si.on_wait = keep
            out.append(inst)
        bb.instructions = out


def _host_prep(inputs, Ts, meta):
    """Build per-core in_maps (numpy only)."""
    COLS = meta["COLS"]
    c_offs = meta["c_offs"]
    NCONST = meta["NCONST"]
    ps = [np.ascontiguousarray(np.asarray(inputs[f"p{l}"], np.float32)) for l in range(NL)]
    layer_shapes = [(p.shape[2], p.shape[3]) for p in ps]

    in_maps = [dict() for _ in range(NCORES)]
    for k in range(NCORES):
        for l in range(NL):
            shard = ps[l][k * BPC : (k + 1) * BPC].reshape(-1, NO)
            in_maps[k][f"p{l}s"] = np.ascontiguousarray(shard)

    n_l = []
    for k in range(NCORES):
        in_maps[k]["posrows"] = np.zeros((P, COLS * NO), np.float32)
        in_maps[k]["ohvals"] = np.zeros((P, COLS), np.float32)
        cst = np.zeros((P, NCONST * COLS), np.float32)
        # benign defaults so padding slots stay finite through the CIoU math
        cst[:, 0 : 2 * COLS] = 1.0  # awh2
        cst[:, 8 * COLS : 9 * COLS] = 1.0  # w2h2pe
        in_maps[k]["consts"] = cst

    for l in range(NL):
        gh, gw = layer_shapes[l]
        b = np.asarray(inputs[f"b{l}"]).astype(np.int64)
        a = np.asarray(inputs[f"a{l}"]).astype(np.int64)
        gj = np.asarray(inputs[f"gj{l}"]).astype(np.int64)
        gi = np.asarray(inputs[f"gi{l}"]).astype(np.int64)
        tc = np.asarray(inputs[f"tcls{l}"]).astype(np.int64)
        tb = np.asarray(inputs[f"tbox{l}"], np.float32)
        an = np.asarray(inputs[f"anch{l}"], np.float32)
        n = b.shape[0]
        n_l.append(n)
        # last-occurrence mask over global cells (images disjoint across cores)
        cell = ((b * NA + a) * gh + gj) * gw + gi
        seen = {}
        for r in range(n):
            seen[int(cell[r])] = r
        last = np.zeros(n, bool)
        last[list(seen.values())] = True

        c0 = int(c_offs[l])
        for k in range(NCORES):
            selm = (b // BPC) == k
            idxs = np.nonzero(selm)[0]
            cnt = idxs.shape[0]
            assert cnt <= P * Ts[l], f"layer {l} core {k}: {cnt} > {P * Ts[l]}"
            lb = b[idxs] - k * BPC
            row = ((lb * NA + a[idxs]) * gh + gj[idxs]) * gw + gi[idxs]
            s = np.arange(cnt)
            pp, tt = s % P, s // P
            tcol = c0 + tt
            im = in_maps[k]
            shard = im[f"p{l}s"]
            pr = im["posrows"].reshape(P, COLS, NO)
            pr[pp, tcol] = shard[row]
            im["ohvals"][pp, tcol] = shard[row, 5 + (tc[idxs] - 1)]
            cv = im["consts"].reshape(P, 6, 2 * COLS)  # 4 paired + 2x2 singles
            # paired blocks: o in 0..3 -> [P, COLS, 2]
            def setp(o, cx, cy):
                blk = im["consts"][:, o * 2 * COLS : (o + 1) * 2 * COLS].reshape(
                    P, COLS, 2
                )
                blk[pp, tcol, 0] = cx
                blk[pp, tcol, 1] = cy

            def sets(o, val):
                blk = im["consts"][:, 8 * COLS + o * COLS : 8 * COLS + (o + 1) * COLS]
                blk[pp, tcol] = val

            x2, y2, w2, h2 = tb[idxs, 0], tb[idxs, 1], tb[idxs, 2], tb[idxs, 3]
            setp(0, 2.0 * an[idxs, 0], 2.0 * an[idxs, 1])
            setp(1, x2 - w2 * 0.5, y2 - h2 * 0.5)
            setp(2, x2 + w2 * 0.5, y2 + h2 * 0.5)
            setp(3, x2, y2)
            sets(0, w2 * h2 + np.float32(EPS))
            sets(1, np.arctan(w2 / (h2 + np.float32(EPS))))
            sets(2, 1.0)
            sets(3, last[idxs].astype(np.float32))
    return in_maps, n_l, layer_shapes


def _combine(outs, n_l, layer_shapes, meta):
    """Host-side reduction of the 8 per-core [P, OUTC] partial tiles."""
    tot = np.zeros(meta["OUTC"], np.float64)
    for o in outs:
        tot += o.astype(np.float64).sum(axis=0)
    lbox = lobj = lcls = 0.0
    for l in range(NL):
        gh, gw = layer_shapes[l]
        G = B * NA * gh * gw
        box = tot[meta["col_box"][l]]
        corr = tot[meta["col_corr"][l]]
        clssp = tot[meta["col_clssp"][l]]
        oh = tot[meta["col_oh"][l]]
        grid = sum(tot[c] for c in meta["grid_cols"][l])
        lbox += box / n_l[l]
        lobj += BALANCE[l] * (grid - corr) / G
        lcls += (clssp - oh) / (n_l[l] * NCLS)
    loss = (HYP_BOX * lbox + HYP_OBJ * lobj + HYP_CLS * lcls) * B
    return np.float32(loss)


def _get_program(inputs):
    ps = [np.asarray(inputs[f"p{l}"]) for l in range(NL)]
    layer_shapes = [(p.shape[2], p.shape[3]) for p in ps]
    # padded slot columns per layer from the worst-case per-core count
    Ts = []
    for l in range(NL):
        b = np.asarray(inputs[f"b{l}"]).astype(np.int64)
        mx = max(int(((b // BPC) == k).sum()) for k in range(NCORES))
        Ts.append(max(1, -(-mx // P)))
    key = (tuple(layer_shapes), tuple(Ts), GRID_MODE)
    if key not in _cache:
        _cache[key] = _build_program(layer_shapes, Ts, GRID_MODE)
    return _cache[key], Ts


last_result = None  # BassKernelResults of the most recent run (for profiling)


def kernel(**inputs) -> np.ndarray:
    global last_result
    (nc, meta), Ts = _get_program(inputs)
    in_maps, n_l, layer_shapes = _host_prep(inputs, Ts, meta)
    from concourse.bass_utils import run_bass_kernel_spmd

    trace = bool(int(os.environ.get("DETLOSS_TRACE", "0")))
    if trace:
        # NTFF profiling needs an initialized PJRT client in this
        # interpreter; warm up with an untraced run first.
        run_bass_kernel_spmd(nc, in_maps, list(range(NCORES)))
    res = run_bass_kernel_spmd(nc, in_maps, list(range(NCORES)), trace=trace)
    last_result = res
    outs = [res.results[k]["partial"] for k in range(NCORES)]
    return _combine(outs, n_l, layer_shapes, meta)



# revision 3
# speedup vs baseline: 2.0393x; 2.0393x over previous
"""YOLOv5 detection-loss (DetLoss) Trainium2 Bass kernel, 8-core SPMD.

Strategy
--------
The loss decomposes so that the only dense work over the big prediction
tensors p0/p1/p2 is a softplus-sum over channel 4 (the objectness logit):

    mean(BCE(x, tobj)) = [ sum_grid softplus(x) - sum_pos tobj_cell * x_cell ] / G

(BCE(x,t) - BCE(x,0) = -t*x, and BCE(x,0) = softplus(x)).  Likewise the
class loss reduces to sum softplus(pcls) - sum pcls[row, tcls-1] over the
gathered positive rows.  The box (CIoU) loss only needs the gathered
positive rows.

Sharding: data-parallel over batch; core k owns images [2k, 2k+2) of
every layer and the positive rows whose image id falls in that range.
Host-side input prep (the same class of layout transform as the
positive-row gather) packs each core's objectness logits into a
contiguous [128, OBJC] plane, so the device reads ~0.2 MB contiguously
instead of 50,400 4-byte strided DMA elements (which are descriptor-rate
bound at ~36 us/core on TRN2's 16 SDMA engines - measured).  The device
computes everything: sigmoid, the full CIoU pipeline, softplus sums over
the whole grid plane and the positive class logits, and the per-layer
partial reductions.  Each core writes a [128, OUTC] tile of partial
sums; the host reduces over partitions and cores and applies the loss
weights.
"""

import os
import numpy as np

# ---------------- problem constants (YOLOv5s / COCO head) ----------------
B, NA, NCLS, NO = 16, 3, 80, 85
NL = 3
NCORES = 8
BPC = B // NCORES  # images per core
BALANCE = (4.0, 1.0, 0.4)
HYP_BOX, HYP_OBJ, HYP_CLS = 0.05, 1.0, 0.05
EPS = 1e-7
P = 128  # SBUF partitions
PAD_NEG = -40.0  # softplus(-40) ~ 4e-18: grid-plane padding value
NCONST = 12

_cache: dict = {}


def _build_program(layer_shapes, T, ocols):
    """Build the SPMD Bass program.

    layer_shapes: [(gh, gw)] * 3; T: padded slot-columns per layer
    (uniform); ocols: per-layer objectness-plane columns.
    Returns (nc, meta) with the accumulator column map.
    """
    import concourse.bass as bass
    import concourse.mybir as mybir
    import concourse.tile as tile

    f32 = mybir.dt.float32
    ALU = mybir.AluOpType
    ACTF = mybir.ActivationFunctionType
    X = mybir.AxisListType.X
    COLS = NL * T
    OBJC = sum(ocols)
    o_offs = np.concatenate([[0], np.cumsum(ocols)]).astype(int)

    nc = bass.Bass()

    POSR = nc.declare_dram_parameter("posrows", [P, COLS * NO], f32, isOutput=False)
    OHV = nc.declare_dram_parameter("ohvals", [P, COLS], f32, isOutput=False)
    # consts layout per partition: 4 paired blocks [COLS,2] (awh2, b2min,
    # b2max, cxy2) then 4 single blocks [COLS] (w2h2pe, atan2c, wbox, wdedup)
    CST = nc.declare_dram_parameter("consts", [P, NCONST * COLS], f32, isOutput=False)
    OBJ = nc.declare_dram_parameter("objplane", [P, OBJC], f32, isOutput=False)

    # accumulator column map
    col_box, col_corr, col_oh, col_cls = 0, 3, 6, 9
    col_grid = 12
    OUTC = 15
    OUT = nc.declare_dram_parameter("partial", [P, OUTC], f32, isOutput=True)

    with tile.TileContext(nc) as tc:
        with (
            tc.tile_pool(name="sp", bufs=2) as sppool,
            tc.tile_pool(name="small", bufs=1) as sm,
        ):
            # ---------- input loads (posrows first: it gates the chain) ----
            pos = sm.tile([P, COLS * NO], f32)
            nc.sync.dma_start(out=pos[:], in_=POSR[:])
            cst = sm.tile([P, NCONST * COLS], f32)
            nc.sync.dma_start(out=cst[:], in_=CST[:])
            ohg = sm.tile([P, COLS], f32)
            nc.sync.dma_start(out=ohg[:], in_=OHV[:])
            obj = sm.tile([P, OBJC], f32)
            nc.sync.dma_start(out=obj[:], in_=OBJ[:])

            acc = sm.tile([P, OUTC], f32)
            nc.vector.memset(acc[:], 0.0)

            pos_v = pos[:].rearrange("p (t c) -> p t c", c=NO)

            # const views
            def paired(o):  # -> [P, COLS, 2]
                return cst[:, o * 2 * COLS : (o + 1) * 2 * COLS].rearrange(
                    "p (t c) -> p t c", c=2
                )

            def single(o):  # -> [P, COLS]
                return cst[:, 8 * COLS + o * COLS : 8 * COLS + (o + 1) * COLS]

            awh2, b2min, b2max, cxy2 = paired(0), paired(1), paired(2), paired(3)
            w2h2pe, atan2c, wbox, wdedup = single(0), single(1), single(2), single(3)

            # ---------- ACT phase 1: sigmoid + arctan (one table set) ------
            sig = sm.tile([P, COLS * 4], f32)
            sig_v = sig[:].rearrange("p (t c) -> p t c", c=4)
            nc.scalar.activation(out=sig_v[:], in_=pos_v[:, :, 0:4], func=ACTF.Sigmoid)

            _tn = [0]

            def pair_tile():
                _tn[0] += 1
                return sm.tile([P, COLS * 2], f32, name=f"pair{_tn[0]}")

            def pv(t):  # view [P, COLS, 2]
                return t[:].rearrange("p (t c) -> p t c", c=2)

            def stile():
                _tn[0] += 1
                return sm.tile([P, COLS], f32, name=f"s{_tn[0]}")

            V = nc.vector
            pxy, swh2, pwhh = pair_tile(), pair_tile(), pair_tile()
            b1min, b1max = pair_tile(), pair_tile()
            tmpa, tmpb = pair_tile(), pair_tile()

            V.tensor_scalar(pv(pxy)[:], sig_v[:, :, 0:2], 2.0, -0.5, ALU.mult, ALU.add)
            V.tensor_tensor(pv(swh2)[:], sig_v[:, :, 2:4], sig_v[:, :, 2:4], ALU.mult)
            V.tensor_tensor(pv(pwhh)[:], pv(swh2)[:], awh2[:], ALU.mult)
            V.tensor_tensor(pv(b1min)[:], pv(pxy)[:], pv(pwhh)[:], ALU.subtract)
            V.tensor_tensor(pv(b1max)[:], pv(pxy)[:], pv(pwhh)[:], ALU.add)

            # intersection
            V.tensor_tensor(pv(tmpa)[:], pv(b1max)[:], b2max[:], ALU.min)
            V.tensor_tensor(pv(tmpb)[:], pv(b1min)[:], b2min[:], ALU.max)
            V.tensor_tensor(pv(tmpa)[:], pv(tmpa)[:], pv(tmpb)[:], ALU.subtract)
            V.tensor_scalar(pv(tmpa)[:], pv(tmpa)[:], 0.0, None, ALU.max)  # relu
            inter = stile()
            V.tensor_tensor(inter[:], pv(tmpa)[:, :, 0], pv(tmpa)[:, :, 1], ALU.mult)
            # union (w1h1 = 4 * pwhh_x * pwhh_y; w2h2pe has +eps folded in)
            u, ru, iou = stile(), stile(), stile()
            V.tensor_tensor(u[:], pv(pwhh)[:, :, 0], pv(pwhh)[:, :, 1], ALU.mult)
            V.tensor_scalar(u[:], u[:], 4.0, None, ALU.mult)
            V.tensor_tensor(u[:], u[:], w2h2pe[:], ALU.add)
            V.tensor_tensor(u[:], u[:], inter[:], ALU.subtract)
            V.reciprocal(ru[:], u[:])
            V.tensor_tensor(iou[:], inter[:], ru[:], ALU.mult)
            # enclosing box diag^2 (c2 > 0 strictly since pwhh > 0: eps drop)
            V.tensor_tensor(pv(tmpa)[:], pv(b1max)[:], b2max[:], ALU.max)
            V.tensor_tensor(pv(tmpb)[:], pv(b1min)[:], b2min[:], ALU.min)
            V.tensor_tensor(pv(tmpa)[:], pv(tmpa)[:], pv(tmpb)[:], ALU.subtract)
            V.tensor_tensor(pv(tmpa)[:], pv(tmpa)[:], pv(tmpa)[:], ALU.mult)
            c2, rc2 = stile(), stile()
            V.tensor_tensor(c2[:], pv(tmpa)[:, :, 0], pv(tmpa)[:, :, 1], ALU.add)
            V.reciprocal(rc2[:], c2[:])
            # center distance^2
            V.tensor_tensor(pv(tmpb)[:], pv(pxy)[:], cxy2[:], ALU.subtract)
            V.tensor_tensor(pv(tmpb)[:], pv(tmpb)[:], pv(tmpb)[:], ALU.mult)
            rho2, rr = stile(), stile()
            V.tensor_tensor(rho2[:], pv(tmpb)[:, :, 0], pv(tmpb)[:, :, 1], ALU.add)
            V.tensor_tensor(rr[:], rho2[:], rc2[:], ALU.mult)
            # v-term: atan(w2/(h2+eps)) - atan(w1/(h1+eps));
            # w1/(h1+eps) == pwhh_x/(pwhh_y+eps/2)
            denh, q = stile(), stile()
            V.tensor_scalar(denh[:], pv(pwhh)[:, :, 1], EPS * 0.5, None, ALU.add)
            V.reciprocal(denh[:], denh[:])
            V.tensor_tensor(q[:], pv(pwhh)[:, :, 0], denh[:], ALU.mult)
            # ACT Arctan only supports [-pi/2, pi/2]; q > 0, so use
            # atan(q) = pi/2 - atan(1/q) for q > 1 (branchless select).
            rq, qm, at, mgt, u2 = stile(), stile(), stile(), stile(), stile()
            V.reciprocal(rq[:], q[:])
            V.tensor_tensor(qm[:], q[:], rq[:], ALU.min)
            i_at = nc.scalar.activation(out=at[:], in_=qm[:], func=ACTF.Arctan)
            V.tensor_scalar(mgt[:], q[:], 1.0, None, ALU.is_gt)
            V.tensor_scalar(u2[:], at[:], -2.0, float(np.pi / 2), ALU.mult, ALU.add)
            V.tensor_tensor(u2[:], mgt[:], u2[:], ALU.mult)
            V.tensor_tensor(at[:], at[:], u2[:], ALU.add)
            dat, v4 = stile(), stile()
            V.tensor_tensor(dat[:], atan2c[:], at[:], ALU.subtract)
            V.tensor_tensor(v4[:], dat[:], dat[:], ALU.mult)
            V.tensor_scalar(v4[:], v4[:], float(4.0 / np.pi**2), None, ALU.mult)
            ad, rad, alpha, va = stile(), stile(), stile(), stile()
            V.tensor_scalar(ad[:], iou[:], -1.0, 1.0 + EPS, ALU.mult, ALU.add)
            V.tensor_tensor(ad[:], ad[:], v4[:], ALU.add)
            V.reciprocal(rad[:], ad[:])
            V.tensor_tensor(alpha[:], v4[:], rad[:], ALU.mult)
            V.tensor_tensor(va[:], v4[:], alpha[:], ALU.mult)
            ciou = stile()
            V.tensor_tensor(ciou[:], iou[:], rr[:], ALU.subtract)
            V.tensor_tensor(ciou[:], ciou[:], va[:], ALU.subtract)

            # per-layer reductions from the ciou tile
            omc, rel, rp4 = stile(), stile(), stile()
            V.tensor_scalar(omc[:], ciou[:], -1.0, 1.0, ALU.mult, ALU.add)
            V.tensor_scalar(rel[:], ciou[:], 0.0, None, ALU.max)
            V.tensor_tensor(rp4[:], rel[:], pos_v[:, :, 4], ALU.mult)

            def lt(tileview):  # [P, COLS] -> [P, NL, T]
                return tileview.rearrange("p (l t) -> p l t", l=NL)

            boxm, corrm = stile(), stile()
            V.tensor_tensor(boxm[:], omc[:], wbox[:], ALU.mult)
            V.reduce_sum(acc[:, col_box : col_box + NL], lt(boxm[:]), X)
            V.tensor_tensor(corrm[:], rp4[:], wdedup[:], ALU.mult)
            V.reduce_sum(acc[:, col_corr : col_corr + NL], lt(corrm[:]), X)
            # ohvals padding is zero: no mask needed
            V.reduce_sum(acc[:, col_oh : col_oh + NL], lt(ohg[:]), X)

            # ---------- ACT phase 2: softplus = ln(1 + exp(x)) -------------
            # (this compiler's table set lacks softplus; exp and ln share
            # natural_log_exp_and_others, so phase 2 costs one table load)
            sp_acts = []
            # class logits: one exp over [P, COLS, 80], one ln, DVE reduce
            # (padding slots are zero -> ln 2 each; host subtracts them)
            spe = sppool.tile([P, COLS * NCLS], f32, tag="spclse")
            spc = sppool.tile([P, COLS * NCLS], f32, tag="spcls")
            i1 = nc.scalar.activation(
                out=spe[:].rearrange("p (t c) -> p t c", c=NCLS),
                in_=pos_v[:, :, 5:NO],
                func=ACTF.Exp,
            )
            i2 = nc.scalar.activation(out=spc[:], in_=spe[:], func=ACTF.Ln, bias=1.0)
            sp_acts += [i1, i2]
            V.reduce_sum(
                acc[:, col_cls : col_cls + NL],
                spc[:].rearrange("p (l t) -> p l t", l=NL),
                X,
            )

            # grid objectness: one exp over the packed plane, per-layer ln
            # with the ACT accumulator (keeps DVE free)
            spge = sppool.tile([P, OBJC], f32, tag="spge")
            i1 = nc.scalar.activation(out=spge[:], in_=obj[:], func=ACTF.Exp)
            sp_acts.append(i1)
            for l in range(NL):
                o0, o1 = int(o_offs[l]), int(o_offs[l + 1])
                spg = sppool.tile([P, ocols[l]], f32, tag="spg")
                i = nc.scalar.activation(
                    out=spg[:, : ocols[l]],
                    in_=spge[:, o0:o1],
                    func=ACTF.Ln,
                    bias=1.0,
                    accum_out=acc[:, col_grid + l : col_grid + l + 1],
                )
                sp_acts.append(i)

            # enforce ACT ordering: all softplus after arctan (2 table loads
            # total: sigmoid_and_others once, ln/exp_and_others once)
            for i in sp_acts:
                tile.add_dep_helper(i.ins, i_at.ins, False, "act-table-grouping")

            # ---------- store partials ----------
            # joiner copy: the store would otherwise wait on ACT + DVE + a DMA
            # lane (3 sems); the copy folds ACT/DVE into one DVE tick.
            acc2 = sm.tile([P, OUTC], f32)
            V.tensor_copy(acc2[:], acc[:])
            nc.sync.dma_start(out=OUT[:], in_=acc2[:])

    _cap_sync_waits(nc, mybir)
    nc.finalize()
    meta = dict(
        COLS=COLS,
        T=T,
        OUTC=OUTC,
        ocols=ocols,
        col_box=col_box,
        col_corr=col_corr,
        col_oh=col_oh,
        col_cls=col_cls,
        col_grid=col_grid,
    )
    return nc, meta


def _cap_sync_waits(nc, mybir, maxw=1):
    """Compute-engine ISA encodings carry very few sync waits; Tile's
    scheduler can emit more (one per DMA sem lane).  Three rewrites, all
    semantics-preserving:
      1. drop waits on the instruction's own engine-completion semaphore
         (engine program order already guarantees them);
      2. hoist waits beyond `maxw` onto standalone EventSemaphore
         instructions placed just before the offender on the same engine;
      3. expand epilogue RANGE_CLEAR (this walrus build can't codegen it)
         into per-semaphore resets, but ONLY for semaphores the program
         actually touches - the full 0..255 sweep costs ~10.5 us of
         serial EventSemaphore instructions at ~140 ns each.
    """
    eng_sem = {
        "DVE": "DVE",
        "Activation": "Activation",
        "SP": "SP",
        "Pool": "Pool",
        "PE": "PE",
    }
    rc_opcode = 176  # NEURON_ISA_TPB_OPCODE_EVENT_SEMAPHORE_RANGE_CLEAR

    # pass 1: collect sem names and the set of sems the program touches
    sem_names = {}
    used = set()
    for bb in nc.m.functions[0].blocks:
        for inst in bb.instructions:
            if (
                type(inst).__name__ == "InstISA"
                and getattr(inst, "isa_opcode", None) == rc_opcode
            ):
                continue
            si = getattr(inst, "sync_info", None)
            if not si:
                continue
            for w in si.on_wait or []:
                sem_names[w.id] = w.ant_name
                used.add(w.id)
            for u in si.on_update or []:
                sem_names[u.id] = u.ant_name
                used.add(u.id)

    n = 0
    for bb in nc.m.functions[0].blocks:
        out = []
        for inst in bb.instructions:
            tname = type(inst).__name__
            if tname == "InstISA" and getattr(inst, "isa_opcode", None) == rc_opcode:
                start, end = inst.instr[13], inst.instr[14]
                for sid in range(start, end + 1):
                    if sid not in used:
                        continue
                    out.append(
                        mybir.InstEventSemaphore(
                            name=f"W-semreset-{sid}",
                            engine=inst.engine,
                            sync_info=mybir.SyncInfo(
                                on_wait=[],
                                on_update=[
                                    mybir.SyncUpdate(
                                        sync_type="semaphore",
                                        id=sid,
                                        update_mode="sem-wr-imm",
                                        update_value=0,
                                        ant_name=sem_names.get(sid, f"sem{sid}"),
                                    )
                                ],
                            ),
                        )
                    )
                continue
            si = getattr(inst, "sync_info", None)
            ow = list(si.on_wait) if (si and si.on_wait) else []
            if ow and tname != "InstEventSemaphore":
                epfx = eng_sem.get(str(inst.engine).split(".")[-1])
                if epfx:
                    keep0 = [
                        w for w in ow if not (w.ant_name or "").startswith(epfx + "_")
                    ]
                else:
                    keep0 = ow
                if len(keep0) > maxw:
                    excess, keep = keep0[:-maxw], keep0[-maxw:]
                    for w in excess:
                        n += 1
                        out.append(
                            mybir.InstEventSemaphore(
                                name=f"W-cap-{n}",
                                engine=inst.engine,
                                sync_info=mybir.SyncInfo(on_wait=[w], on_update=[]),
                            )
                        )
                else:
                    keep = keep0
                if len(keep) != len(ow):
                    si.on_wait = keep
            out.append(inst)
        bb.instructions = out


def _host_prep(inputs, T, ocols, meta):
    """Build per-core in_maps (numpy only)."""
    COLS = meta["COLS"]
    ps = [np.asarray(inputs[f"p{l}"]) for l in range(NL)]
    layer_shapes = [(p.shape[2], p.shape[3]) for p in ps]

    in_maps = [dict() for _ in range(NCORES)]
    for k in range(NCORES):
        in_maps[k]["posrows"] = np.zeros((P, COLS * NO), np.float32)
        in_maps[k]["ohvals"] = np.zeros((P, COLS), np.float32)
        cst = np.zeros((P, NCONST * COLS), np.float32)
        # benign defaults so padding slots stay finite through the CIoU math
        cst[:, 0 : 2 * COLS] = 1.0  # awh2
        cst[:, 8 * COLS : 9 * COLS] = 1.0  # w2h2pe
        in_maps[k]["consts"] = cst
        # packed objectness plane (all layers, padded with PAD_NEG)
        plane = np.full(P * sum(ocols), PAD_NEG, np.float32)
        pm = plane.reshape(P, sum(ocols))
        o0 = 0
        for l in range(NL):
            ch4 = np.ascontiguousarray(
                ps[l][k * BPC : (k + 1) * BPC, :, :, :, 4], np.float32
            ).reshape(-1)
            cells = ch4.shape[0]
            buf = np.full(P * ocols[l], PAD_NEG, np.float32)
            buf[:cells] = ch4
            pm[:, o0 : o0 + ocols[l]] = buf.reshape(P, ocols[l])
            o0 += ocols[l]
        in_maps[k]["objplane"] = pm

    n_l = []
    pad_slots = 0  # padded positive slots across layers/cores (for lcls)
    for l in range(NL):
        gh, gw = layer_shapes[l]
        flat = ps[l].reshape(-1, NO)  # view, no copy
        rows_per_img = NA * gh * gw
        b = np.asarray(inputs[f"b{l}"]).astype(np.int64)
        a = np.asarray(inputs[f"a{l}"]).astype(np.int64)
        gj = np.asarray(inputs[f"gj{l}"]).astype(np.int64)
        gi = np.asarray(inputs[f"gi{l}"]).astype(np.int64)
        tc = np.asarray(inputs[f"tcls{l}"]).astype(np.int64)
        tb = np.asarray(inputs[f"tbox{l}"], np.float32)
        an = np.asarray(inputs[f"anch{l}"], np.float32)
        n = b.shape[0]
        n_l.append(n)
        # last-occurrence mask over global cells (images disjoint across cores)
        cell = ((b * NA + a) * gh + gj) * gw + gi
        seen = {}
        for r in range(n):
            seen[int(cell[r])] = r
        last = np.zeros(n, bool)
        last[list(seen.values())] = True

        c0 = l * T
        for k in range(NCORES):
            idxs = np.nonzero((b // BPC) == k)[0]
            cnt = idxs.shape[0]
            assert cnt <= P * T, f"layer {l} core {k}: {cnt} > {P * T}"
            pad_slots += P * T - cnt
            row = b[idxs] * rows_per_img + (
                (a[idxs] * gh + gj[idxs]) * gw + gi[idxs]
            )
            s = np.arange(cnt)
            pp, tcol = s % P, c0 + s // P
            im = in_maps[k]
            pr = im["posrows"].reshape(P, COLS, NO)
            pr[pp, tcol] = flat[row]
            im["ohvals"][pp, tcol] = flat[row, 5 + (tc[idxs] - 1)]

            def setp(o, cx, cy):
                blk = im["consts"][:, o * 2 * COLS : (o + 1) * 2 * COLS].reshape(
                    P, COLS, 2
                )
                blk[pp, tcol, 0] = cx
                blk[pp, tcol, 1] = cy

            def sets(o, val):
                blk = im["consts"][:, 8 * COLS + o * COLS : 8 * COLS + (o + 1) * COLS]
                blk[pp, tcol] = val

            x2, y2, w2, h2 = tb[idxs, 0], tb[idxs, 1], tb[idxs, 2], tb[idxs, 3]
            setp(0, 2.0 * an[idxs, 0], 2.0 * an[idxs, 1])
            setp(1, x2 - w2 * 0.5, y2 - h2 * 0.5)
            setp(2, x2 + w2 * 0.5, y2 + h2 * 0.5)
            setp(3, x2, y2)
            sets(0, w2 * h2 + np.float32(EPS))
            sets(1, np.arctan(w2 / (h2 + np.float32(EPS))))
            sets(2, 1.0)
            sets(3, last[idxs].astype(np.float32))
    return in_maps, n_l, pad_slots, layer_shapes


def _combine(outs, n_l, pad_slots, layer_shapes, meta):
    """Host-side reduction of the 8 per-core [P, OUTC] partial tiles."""
    tot = np.zeros(meta["OUTC"], np.float64)
    for o in outs:
        tot += o.astype(np.float64).sum(axis=0)
    lbox = lobj = lcls = 0.0
    # padded positive slots contribute softplus(0) = ln 2 per class logit
    for l in range(NL):
        gh, gw = layer_shapes[l]
        G = B * NA * gh * gw
        box = tot[meta["col_box"] + l]
        corr = tot[meta["col_corr"] + l]
        oh = tot[meta["col_oh"] + l]
        clssp = tot[meta["col_cls"] + l] - meta["pad_l"][l] * NCLS * np.log(2.0)
        grid = tot[meta["col_grid"] + l]
        lbox += box / n_l[l]
        lobj += BALANCE[l] * (grid - corr) / G
        lcls += (clssp - oh) / (n_l[l] * NCLS)
    loss = (HYP_BOX * lbox + HYP_OBJ * lobj + HYP_CLS * lcls) * B
    return np.float32(loss)


def _get_program(inputs):
    ps = [np.asarray(inputs[f"p{l}"]) for l in range(NL)]
    layer_shapes = [(p.shape[2], p.shape[3]) for p in ps]
    # padded slot columns (uniform across layers) from worst-case per-core
    T = 1
    for l in range(NL):
        b = np.asarray(inputs[f"b{l}"]).astype(np.int64)
        mx = max(int(((b // BPC) == k).sum()) for k in range(NCORES))
        T = max(T, -(-mx // P))
    ocols = tuple(
        -(-(BPC * NA * gh * gw) // P) for gh, gw in layer_shapes
    )
    key = (tuple(layer_shapes), T, ocols)
    if key not in _cache:
        _cache[key] = _build_program(layer_shapes, T, ocols)
    return _cache[key], T, ocols


last_result = None  # BassKernelResults of the most recent run (for profiling)


def kernel(**inputs) -> np.ndarray:
    global last_result
    (nc, meta), T, ocols = _get_program(inputs)
    in_maps, n_l, pad_slots, layer_shapes = _host_prep(inputs, T, ocols, meta)
    # per-layer padded-slot counts for the lcls correction
    meta["pad_l"] = [P * T * NCORES - n_l[l] for l in range(NL)]
    from concourse.bass_utils import run_bass_kernel_spmd

    trace = bool(int(os.environ.get("DETLOSS_TRACE", "0")))
    if trace:
        # NTFF profiling needs an initialized PJRT client in this
        # interpreter; warm up with an untraced run first.
        run_bass_kernel_spmd(nc, in_maps, list(range(NCORES)))
    res = run_bass_kernel_spmd(nc, in_maps, list(range(NCORES)), trace=trace)
    last_result = res
    outs = [res.results[k]["partial"] for k in range(NCORES)]
    return _combine(outs, n_l, pad_slots, layer_shapes, meta)


# revision 9
# speedup vs baseline: 2.0526x; 1.0065x over previous
"""YOLOv5 detection-loss (DetLoss) Trainium2 Bass kernel, 8-core SPMD.

Strategy
--------
The loss decomposes so that the only dense work over the big prediction
tensors p0/p1/p2 is a softplus-sum over channel 4 (the objectness logit):

    mean(BCE(x, tobj)) = [ sum_grid softplus(x) - sum_pos tobj_cell * x_cell ] / G

(BCE(x,t) - BCE(x,0) = -t*x, and BCE(x,0) = softplus(x)).  Likewise the
class loss reduces to sum softplus(pcls) - sum pcls[row, tcls-1] over the
gathered positive rows.  The box (CIoU) loss only needs the gathered
positive rows.

Sharding: data-parallel over batch; core k owns images [2k, 2k+2) of
every layer and the positive rows whose image id falls in that range.
Host-side input prep (the same class of layout transform as the
positive-row gather) packs each core's objectness logits into a
contiguous [128, OBJC] plane, so the device reads ~0.2 MB contiguously
instead of 50,400 4-byte strided DMA elements (which are descriptor-rate
bound at ~36 us/core on TRN2's 16 SDMA engines - measured).  The device
computes everything: sigmoid, the full CIoU pipeline, softplus sums over
the whole grid plane and the positive class logits, and the per-layer
partial reductions.  Each core writes a [128, OUTC] tile of partial
sums; the host reduces over partitions and cores and applies the loss
weights.
"""

import os
import numpy as np

# ---------------- problem constants (YOLOv5s / COCO head) ----------------
B, NA, NCLS, NO = 16, 3, 80, 85
NL = 3
NCORES = 8
BPC = B // NCORES  # images per core
BALANCE = (4.0, 1.0, 0.4)
HYP_BOX, HYP_OBJ, HYP_CLS = 0.05, 1.0, 0.05
EPS = 1e-7
P = 128  # SBUF partitions
PAD_NEG = -40.0  # softplus(-40) ~ 4e-18: grid-plane padding value
NCONST = 12

_cache: dict = {}


def _build_program(layer_shapes, T, ocols):
    """Build the SPMD Bass program.

    layer_shapes: [(gh, gw)] * 3; T: padded slot-columns per layer
    (uniform); ocols: per-layer objectness-plane columns.
    Returns (nc, meta) with the accumulator column map.
    """
    import concourse.bass as bass
    import concourse.mybir as mybir
    import concourse.tile as tile

    f32 = mybir.dt.float32
    ALU = mybir.AluOpType
    ACTF = mybir.ActivationFunctionType
    X = mybir.AxisListType.X
    COLS = NL * T
    OBJC = sum(ocols)
    o_offs = np.concatenate([[0], np.cumsum(ocols)]).astype(int)

    nc = bass.Bass()

    # two merged input tensors: [posrows | consts] (gates the compute
    # chain) and [ohvals | objplane] (needed later) - one DMA each, so the
    # first data is in SBUF ~1 us earlier than with four separate loads.
    # consts layout per partition: 4 paired blocks [COLS,2] (awh2, b2min,
    # b2max, cxy2) then 4 single blocks [COLS] (w2h2pe, atan2c, wbox, wdedup)
    PCW = COLS * NO + NCONST * COLS
    OOW = COLS + OBJC
    PC = nc.declare_dram_parameter("pc", [P, PCW], f32, isOutput=False)
    OO = nc.declare_dram_parameter("oo", [P, OOW], f32, isOutput=False)

    # accumulator column map
    col_box, col_corr, col_oh, col_cls = 0, 3, 6, 9
    col_grid = 12
    OUTC = 15
    OUT = nc.declare_dram_parameter("partial", [P, OUTC], f32, isOutput=True)

    with tile.TileContext(nc) as tc:
        with tc.tile_pool(name="small", bufs=1) as sm:
            # ---------- input loads ----------
            pc = sm.tile([P, PCW], f32)
            nc.sync.dma_start(out=pc[:], in_=PC[:])
            oo = sm.tile([P, OOW], f32)
            nc.sync.dma_start(out=oo[:], in_=OO[:])
            acc = sm.tile([P, OUTC], f32)

            pos_v = pc[:, : COLS * NO].rearrange("p (t c) -> p t c", c=NO)
            _cb = COLS * NO  # consts base inside pc

            def paired(o):  # -> [P, COLS, 2]
                return pc[:, _cb + o * 2 * COLS : _cb + (o + 1) * 2 * COLS].rearrange(
                    "p (t c) -> p t c", c=2
                )

            def single(o):  # -> [P, COLS]
                s = _cb + 8 * COLS + o * COLS
                return pc[:, s : s + COLS]

            ohg = oo[:, :COLS]
            obj = oo[:, COLS:]

            awh2, b2min, b2max, cxy2 = paired(0), paired(1), paired(2), paired(3)
            w2h2pe, atan2c, wbox, wdedup = single(0), single(1), single(2), single(3)

            # ---------- ACT phase 1: sigmoid + arctan (one table set) ------
            sig = sm.tile([P, COLS * 4], f32)
            sig_v = sig[:].rearrange("p (t c) -> p t c", c=4)
            nc.scalar.activation(out=sig_v[:], in_=pos_v[:, :, 0:4], func=ACTF.Sigmoid)

            _tn = [0]

            def pair_tile():
                _tn[0] += 1
                return sm.tile([P, COLS * 2], f32, name=f"pair{_tn[0]}")

            def pv(t):  # view [P, COLS, 2]
                return t[:].rearrange("p (t c) -> p t c", c=2)

            def stile():
                _tn[0] += 1
                return sm.tile([P, COLS], f32, name=f"s{_tn[0]}")

            V = nc.vector
            pxy, swh2, pwhh = pair_tile(), pair_tile(), pair_tile()
            b1min, b1max = pair_tile(), pair_tile()
            tmpa, tmpb = pair_tile(), pair_tile()

            V.tensor_scalar(pv(pxy)[:], sig_v[:, :, 0:2], 2.0, -0.5, ALU.mult, ALU.add)
            V.tensor_tensor(pv(swh2)[:], sig_v[:, :, 2:4], sig_v[:, :, 2:4], ALU.mult)
            V.tensor_tensor(pv(pwhh)[:], pv(swh2)[:], awh2[:], ALU.mult)

            # v-term argument FIRST so the ACT arctan (and with it the
            # exp/ln table switch) unblocks as early as possible:
            # w1/(h1+eps) == pwhh_x/(pwhh_y+eps/2)
            denh, q = stile(), stile()
            V.tensor_scalar(denh[:], pv(pwhh)[:, :, 1], EPS * 0.5, None, ALU.add)
            V.reciprocal(denh[:], denh[:])
            V.tensor_tensor(q[:], pv(pwhh)[:, :, 0], denh[:], ALU.mult)
            # ACT Arctan only supports [-pi/2, pi/2]; q > 0, so use
            # atan(q) = pi/2 - atan(1/q) for q > 1 (branchless select).
            rq, qm, at, mgt, u2 = stile(), stile(), stile(), stile(), stile()
            V.reciprocal(rq[:], q[:])
            V.tensor_tensor(qm[:], q[:], rq[:], ALU.min)
            i_at = nc.scalar.activation(out=at[:], in_=qm[:], func=ACTF.Arctan)

            V.tensor_tensor(pv(b1min)[:], pv(pxy)[:], pv(pwhh)[:], ALU.subtract)
            V.tensor_tensor(pv(b1max)[:], pv(pxy)[:], pv(pwhh)[:], ALU.add)

            # intersection
            V.tensor_tensor(pv(tmpa)[:], pv(b1max)[:], b2max[:], ALU.min)
            V.tensor_tensor(pv(tmpb)[:], pv(b1min)[:], b2min[:], ALU.max)
            V.tensor_tensor(pv(tmpa)[:], pv(tmpa)[:], pv(tmpb)[:], ALU.subtract)
            V.tensor_scalar(pv(tmpa)[:], pv(tmpa)[:], 0.0, None, ALU.max)  # relu
            inter = stile()
            V.tensor_tensor(inter[:], pv(tmpa)[:, :, 0], pv(tmpa)[:, :, 1], ALU.mult)
            # union (w1h1 = 4 * pwhh_x * pwhh_y; w2h2pe has +eps folded in)
            u, ru, iou = stile(), stile(), stile()
            V.tensor_tensor(u[:], pv(pwhh)[:, :, 0], pv(pwhh)[:, :, 1], ALU.mult)
            V.tensor_scalar(u[:], u[:], 4.0, None, ALU.mult)
            V.tensor_tensor(u[:], u[:], w2h2pe[:], ALU.add)
            V.tensor_tensor(u[:], u[:], inter[:], ALU.subtract)
            V.reciprocal(ru[:], u[:])
            V.tensor_tensor(iou[:], inter[:], ru[:], ALU.mult)
            # enclosing box diag^2 (c2 > 0 strictly since pwhh > 0: eps drop)
            V.tensor_tensor(pv(tmpa)[:], pv(b1max)[:], b2max[:], ALU.max)
            V.tensor_tensor(pv(tmpb)[:], pv(b1min)[:], b2min[:], ALU.min)
            V.tensor_tensor(pv(tmpa)[:], pv(tmpa)[:], pv(tmpb)[:], ALU.subtract)
            V.tensor_tensor(pv(tmpa)[:], pv(tmpa)[:], pv(tmpa)[:], ALU.mult)
            c2, rc2 = stile(), stile()
            V.tensor_tensor(c2[:], pv(tmpa)[:, :, 0], pv(tmpa)[:, :, 1], ALU.add)
            V.reciprocal(rc2[:], c2[:])
            # center distance^2
            V.tensor_tensor(pv(tmpb)[:], pv(pxy)[:], cxy2[:], ALU.subtract)
            V.tensor_tensor(pv(tmpb)[:], pv(tmpb)[:], pv(tmpb)[:], ALU.mult)
            rho2, rr = stile(), stile()
            V.tensor_tensor(rho2[:], pv(tmpb)[:, :, 0], pv(tmpb)[:, :, 1], ALU.add)
            V.tensor_tensor(rr[:], rho2[:], rc2[:], ALU.mult)
            # arctan range correction + v term
            V.tensor_scalar(mgt[:], q[:], 1.0, None, ALU.is_gt)
            V.tensor_scalar(u2[:], at[:], -2.0, float(np.pi / 2), ALU.mult, ALU.add)
            V.tensor_tensor(u2[:], mgt[:], u2[:], ALU.mult)
            V.tensor_tensor(at[:], at[:], u2[:], ALU.add)
            dat, v4 = stile(), stile()
            V.tensor_tensor(dat[:], atan2c[:], at[:], ALU.subtract)
            V.tensor_tensor(v4[:], dat[:], dat[:], ALU.mult)
            V.tensor_scalar(v4[:], v4[:], float(4.0 / np.pi**2), None, ALU.mult)
            ad, rad, alpha, va = stile(), stile(), stile(), stile()
            V.tensor_scalar(ad[:], iou[:], -1.0, 1.0 + EPS, ALU.mult, ALU.add)
            V.tensor_tensor(ad[:], ad[:], v4[:], ALU.add)
            V.reciprocal(rad[:], ad[:])
            V.tensor_tensor(alpha[:], v4[:], rad[:], ALU.mult)
            V.tensor_tensor(va[:], v4[:], alpha[:], ALU.mult)
            ciou = stile()
            V.tensor_tensor(ciou[:], iou[:], rr[:], ALU.subtract)
            V.tensor_tensor(ciou[:], ciou[:], va[:], ALU.subtract)

            # per-layer reductions from the ciou tile
            omc, rel, rp4 = stile(), stile(), stile()
            V.tensor_scalar(omc[:], ciou[:], -1.0, 1.0, ALU.mult, ALU.add)
            V.tensor_scalar(rel[:], ciou[:], 0.0, None, ALU.max)
            V.tensor_tensor(rp4[:], rel[:], pos_v[:, :, 4], ALU.mult)

            def lt(view):  # [P, COLS] -> [P, NL, T]
                return view.rearrange("p (l t) -> p l t", l=NL)

            boxm, corrm = stile(), stile()
            V.tensor_tensor(boxm[:], omc[:], wbox[:], ALU.mult)
            V.reduce_sum(acc[:, col_box : col_box + NL], lt(boxm[:]), X)
            V.tensor_tensor(corrm[:], rp4[:], wdedup[:], ALU.mult)
            V.reduce_sum(acc[:, col_corr : col_corr + NL], lt(corrm[:]), X)
            # ohvals padding is zero: no mask needed
            V.reduce_sum(acc[:, col_oh : col_oh + NL], lt(ohg), X)

            # ---------- ACT phase 2: softplus = ln(1 + exp(x)) -------------
            # (this compiler's table set lacks softplus; exp and ln share
            # natural_log_exp_and_others, so phase 2 costs one table load.)
            # Order: cls exp -> cls ln (gates the 1.4us DVE reduce) ->
            # obj exp -> per-layer obj ln+accum (ACT-local tail).
            sp_acts = []
            # class logits: one exp over [P, COLS, 80], one ln, DVE reduce
            # (padding slots are zero -> ln 2 each; host subtracts them)
            spe = sm.tile([P, COLS * NCLS], f32)
            spc = sm.tile([P, COLS * NCLS], f32)
            i1 = nc.scalar.activation(
                out=spe[:].rearrange("p (t c) -> p t c", c=NCLS),
                in_=pos_v[:, :, 5:NO],
                func=ACTF.Exp,
            )
            i2 = nc.scalar.activation(out=spc[:], in_=spe[:], func=ACTF.Ln, bias=1.0)
            sp_acts += [i1, i2]
            V.reduce_sum(
                acc[:, col_cls : col_cls + NL],
                spc[:].rearrange("p (l t) -> p l t", l=NL),
                X,
            )

            # grid objectness: one exp over the packed plane, per-layer ln
            # with the ACT accumulator (keeps DVE free)
            spge = sm.tile([P, OBJC], f32)
            i1 = nc.scalar.activation(out=spge[:], in_=obj, func=ACTF.Exp)
            sp_acts.append(i1)
            for l in range(NL):
                o0, o1 = int(o_offs[l]), int(o_offs[l + 1])
                spg = sm.tile([P, ocols[l]], f32, name=f"spg{l}")
                i = nc.scalar.activation(
                    out=spg[:, : ocols[l]],
                    in_=spge[:, o0:o1],
                    func=ACTF.Ln,
                    bias=1.0,
                    accum_out=acc[:, col_grid + l : col_grid + l + 1],
                )
                sp_acts.append(i)

            # enforce ACT ordering: all softplus after arctan (2 table loads
            # total: sigmoid_and_others once, ln/exp_and_others once)
            for i in sp_acts:
                tile.add_dep_helper(i.ins, i_at.ins, False, "act-table-grouping")

            # ---------- store partials (wait-cap hoists the extra sems) ----
            nc.sync.dma_start(out=OUT[:], in_=acc[:])

    _cap_sync_waits(nc, mybir)
    nc.finalize()
    meta = dict(
        COLS=COLS,
        T=T,
        OUTC=OUTC,
        ocols=ocols,
        col_box=col_box,
        col_corr=col_corr,
        col_oh=col_oh,
        col_cls=col_cls,
        col_grid=col_grid,
    )
    return nc, meta


def _cap_sync_waits(nc, mybir, maxw=1):
    """Compute-engine ISA encodings carry very few sync waits; Tile's
    scheduler can emit more (one per DMA sem lane).  Three rewrites, all
    semantics-preserving:
      1. drop waits on the instruction's own engine-completion semaphore
         (engine program order already guarantees them);
      2. hoist waits beyond `maxw` onto standalone EventSemaphore
         instructions placed just before the offender on the same engine;
      3. expand epilogue RANGE_CLEAR (this walrus build can't codegen it)
         into per-semaphore resets, but ONLY for semaphores the program
         actually touches - the full 0..255 sweep costs ~10.5 us of
         serial EventSemaphore instructions at ~140 ns each.
    """
    eng_sem = {
        "DVE": "DVE",
        "Activation": "Activation",
        "SP": "SP",
        "Pool": "Pool",
        "PE": "PE",
    }
    rc_opcode = 176  # NEURON_ISA_TPB_OPCODE_EVENT_SEMAPHORE_RANGE_CLEAR

    # pass 1: collect sem names and the set of sems the program touches
    sem_names = {}
    used = set()
    for bb in nc.m.functions[0].blocks:
        for inst in bb.instructions:
            if (
                type(inst).__name__ == "InstISA"
                and getattr(inst, "isa_opcode", None) == rc_opcode
            ):
                continue
            si = getattr(inst, "sync_info", None)
            if not si:
                continue
            for w in si.on_wait or []:
                sem_names[w.id] = w.ant_name
                used.add(w.id)
            for u in si.on_update or []:
                sem_names[u.id] = u.ant_name
                used.add(u.id)

    n = 0
    for bb in nc.m.functions[0].blocks:
        out = []
        for inst in bb.instructions:
            tname = type(inst).__name__
            if tname == "InstISA" and getattr(inst, "isa_opcode", None) == rc_opcode:
                start, end = inst.instr[13], inst.instr[14]
                for sid in range(start, end + 1):
                    if sid not in used:
                        continue
                    out.append(
                        mybir.InstEventSemaphore(
                            name=f"W-semreset-{sid}",
                            engine=inst.engine,
                            sync_info=mybir.SyncInfo(
                                on_wait=[],
                                on_update=[
                                    mybir.SyncUpdate(
                                        sync_type="semaphore",
                                        id=sid,
                                        update_mode="sem-wr-imm",
                                        update_value=0,
                                        ant_name=sem_names.get(sid, f"sem{sid}"),
                                    )
                                ],
                            ),
                        )
                    )
                continue
            si = getattr(inst, "sync_info", None)
            ow = list(si.on_wait) if (si and si.on_wait) else []
            if ow and tname != "InstEventSemaphore":
                epfx = eng_sem.get(str(inst.engine).split(".")[-1])
                if epfx:
                    keep0 = [
                        w for w in ow if not (w.ant_name or "").startswith(epfx + "_")
                    ]
                else:
                    keep0 = ow
                if len(keep0) > maxw:
                    excess, keep = keep0[:-maxw], keep0[-maxw:]
                    for w in excess:
                        n += 1
                        out.append(
                            mybir.InstEventSemaphore(
                                name=f"W-cap-{n}",
                                engine=inst.engine,
                                sync_info=mybir.SyncInfo(on_wait=[w], on_update=[]),
                            )
                        )
                else:
                    keep = keep0
                if len(keep) != len(ow):
                    si.on_wait = keep
            out.append(inst)
        bb.instructions = out


def _host_prep(inputs, T, ocols, meta):
    """Build per-core in_maps (numpy only)."""
    COLS = meta["COLS"]
    ps = [np.asarray(inputs[f"p{l}"]) for l in range(NL)]
    layer_shapes = [(p.shape[2], p.shape[3]) for p in ps]

    OBJC = sum(ocols)
    in_maps = [dict() for _ in range(NCORES)]
    posrows_k, consts_k, ohvals_k = [], [], []
    for k in range(NCORES):
        posrows_k.append(np.zeros((P, COLS * NO), np.float32))
        ohvals_k.append(np.zeros((P, COLS), np.float32))
        cst = np.zeros((P, NCONST * COLS), np.float32)
        # benign defaults so padding slots stay finite through the CIoU math
        cst[:, 0 : 2 * COLS] = 1.0  # awh2
        cst[:, 8 * COLS : 9 * COLS] = 1.0  # w2h2pe
        consts_k.append(cst)
        # [ohvals | objplane] merged input; plane padded with PAD_NEG
        oo = np.empty((P, COLS + OBJC), np.float32)
        in_maps[k]["oo"] = oo
        pm = oo[:, COLS:]
        o0 = 0
        for l in range(NL):
            ch4 = np.ascontiguousarray(
                ps[l][k * BPC : (k + 1) * BPC, :, :, :, 4], np.float32
            ).reshape(-1)
            cells = ch4.shape[0]
            buf = np.full(P * ocols[l], PAD_NEG, np.float32)
            buf[:cells] = ch4
            pm[:, o0 : o0 + ocols[l]] = buf.reshape(P, ocols[l])
            o0 += ocols[l]

    n_l = []
    pad_slots = 0  # padded positive slots across layers/cores (for lcls)
    for l in range(NL):
        gh, gw = layer_shapes[l]
        flat = ps[l].reshape(-1, NO)  # view, no copy
        rows_per_img = NA * gh * gw
        b = np.asarray(inputs[f"b{l}"]).astype(np.int64)
        a = np.asarray(inputs[f"a{l}"]).astype(np.int64)
        gj = np.asarray(inputs[f"gj{l}"]).astype(np.int64)
        gi = np.asarray(inputs[f"gi{l}"]).astype(np.int64)
        tc = np.asarray(inputs[f"tcls{l}"]).astype(np.int64)
        tb = np.asarray(inputs[f"tbox{l}"], np.float32)
        an = np.asarray(inputs[f"anch{l}"], np.float32)
        n = b.shape[0]
        n_l.append(n)
        # last-occurrence mask over global cells (images disjoint across cores)
        cell = ((b * NA + a) * gh + gj) * gw + gi
        seen = {}
        for r in range(n):
            seen[int(cell[r])] = r
        last = np.zeros(n, bool)
        last[list(seen.values())] = True

        c0 = l * T
        for k in range(NCORES):
            idxs = np.nonzero((b // BPC) == k)[0]
            cnt = idxs.shape[0]
            assert cnt <= P * T, f"layer {l} core {k}: {cnt} > {P * T}"
            pad_slots += P * T - cnt
            row = b[idxs] * rows_per_img + (
                (a[idxs] * gh + gj[idxs]) * gw + gi[idxs]
            )
            s = np.arange(cnt)
            pp, tcol = s % P, c0 + s // P
            pr = posrows_k[k].reshape(P, COLS, NO)
            pr[pp, tcol] = flat[row]
            ohvals_k[k][pp, tcol] = flat[row, 5 + (tc[idxs] - 1)]

            def setp(o, cx, cy):
                blk = consts_k[k][:, o * 2 * COLS : (o + 1) * 2 * COLS].reshape(
                    P, COLS, 2
                )
                blk[pp, tcol, 0] = cx
                blk[pp, tcol, 1] = cy

            def sets(o, val):
                blk = consts_k[k][:, 8 * COLS + o * COLS : 8 * COLS + (o + 1) * COLS]
                blk[pp, tcol] = val

            x2, y2, w2, h2 = tb[idxs, 0], tb[idxs, 1], tb[idxs, 2], tb[idxs, 3]
            setp(0, 2.0 * an[idxs, 0], 2.0 * an[idxs, 1])
            setp(1, x2 - w2 * 0.5, y2 - h2 * 0.5)
            setp(2, x2 + w2 * 0.5, y2 + h2 * 0.5)
            setp(3, x2, y2)
            sets(0, w2 * h2 + np.float32(EPS))
            sets(1, np.arctan(w2 / (h2 + np.float32(EPS))))
            sets(2, 1.0)
            sets(3, last[idxs].astype(np.float32))
    for k in range(NCORES):
        in_maps[k]["pc"] = np.hstack([posrows_k[k], consts_k[k]])
        in_maps[k]["oo"][:, :COLS] = ohvals_k[k]
    return in_maps, n_l, pad_slots, layer_shapes


def _combine(outs, n_l, pad_slots, layer_shapes, meta):
    """Host-side reduction of the 8 per-core [P, OUTC] partial tiles."""
    tot = np.zeros(meta["OUTC"], np.float64)
    for o in outs:
        tot += o.astype(np.float64).sum(axis=0)
    lbox = lobj = lcls = 0.0
    # padded positive slots contribute softplus(0) = ln 2 per class logit
    for l in range(NL):
        gh, gw = layer_shapes[l]
        G = B * NA * gh * gw
        box = tot[meta["col_box"] + l]
        corr = tot[meta["col_corr"] + l]
        oh = tot[meta["col_oh"] + l]
        clssp = tot[meta["col_cls"] + l] - meta["pad_l"][l] * NCLS * np.log(2.0)
        grid = tot[meta["col_grid"] + l]
        lbox += box / n_l[l]
        lobj += BALANCE[l] * (grid - corr) / G
        lcls += (clssp - oh) / (n_l[l] * NCLS)
    loss = (HYP_BOX * lbox + HYP_OBJ * lobj + HYP_CLS * lcls) * B
    return np.float32(loss)


def _get_program(inputs):
    ps = [np.asarray(inputs[f"p{l}"]) for l in range(NL)]
    layer_shapes = [(p.shape[2], p.shape[3]) for p in ps]
    # padded slot columns (uniform across layers) from worst-case per-core
    T = 1
    for l in range(NL):
        b = np.asarray(inputs[f"b{l}"]).astype(np.int64)
        mx = max(int(((b // BPC) == k).sum()) for k in range(NCORES))
        T = max(T, -(-mx // P))
    ocols = tuple(
        -(-(BPC * NA * gh * gw) // P) for gh, gw in layer_shapes
    )
    key = (tuple(layer_shapes), T, ocols)
    if key not in _cache:
        _cache[key] = _build_program(layer_shapes, T, ocols)
    return _cache[key], T, ocols


last_result = None  # BassKernelResults of the most recent run (for profiling)


def kernel(**inputs) -> np.ndarray:
    global last_result
    (nc, meta), T, ocols = _get_program(inputs)
    in_maps, n_l, pad_slots, layer_shapes = _host_prep(inputs, T, ocols, meta)
    # per-layer padded-slot counts for the lcls correction
    meta["pad_l"] = [P * T * NCORES - n_l[l] for l in range(NL)]
    from concourse.bass_utils import run_bass_kernel_spmd

    trace = bool(int(os.environ.get("DETLOSS_TRACE", "0")))
    if trace:
        # NTFF profiling needs an initialized PJRT client in this
        # interpreter; warm up with an untraced run first.
        run_bass_kernel_spmd(nc, in_maps, list(range(NCORES)))
    res = run_bass_kernel_spmd(nc, in_maps, list(range(NCORES)), trace=trace)
    last_result = res
    outs = [res.results[k]["partial"] for k in range(NCORES)]
    return _combine(outs, n_l, pad_slots, layer_shapes, meta)


# revision 14
# speedup vs baseline: 2.2439x; 1.0932x over previous
"""YOLOv5 detection-loss (DetLoss) Trainium2 Bass kernel, 8-core SPMD.

Strategy
--------
The loss decomposes so that the only dense work over the big prediction
tensors p0/p1/p2 is a softplus-sum over channel 4 (the objectness logit):

    mean(BCE(x, tobj)) = [ sum_grid softplus(x) - sum_pos tobj_cell * x_cell ] / G

(BCE(x,t) - BCE(x,0) = -t*x, and BCE(x,0) = softplus(x)).  Likewise the
class loss reduces to sum softplus(pcls) - sum pcls[row, tcls-1] over the
gathered positive rows.  The box (CIoU) loss only needs the gathered
positive rows.

Sharding: data-parallel over batch; core k owns images [2k, 2k+2) of
every layer and the positive rows whose image id falls in that range.
Host-side input prep (the same class of layout transform as the
positive-row gather) packs each core's objectness logits into a
contiguous [128, OBJC] plane, so the device reads ~0.2 MB contiguously
instead of 50,400 4-byte strided DMA elements (which are descriptor-rate
bound at ~36 us/core on TRN2's 16 SDMA engines - measured).  The device
computes everything: sigmoid, the full CIoU pipeline, softplus sums over
the whole grid plane and the positive class logits, and the per-layer
partial reductions.  Each core writes a [128, OUTC] tile of partial
sums; the host reduces over partitions and cores and applies the loss
weights.
"""

import os
import numpy as np

# ---------------- problem constants (YOLOv5s / COCO head) ----------------
B, NA, NCLS, NO = 16, 3, 80, 85
NL = 3
NCORES = 8
BPC = B // NCORES  # images per core
BALANCE = (4.0, 1.0, 0.4)
HYP_BOX, HYP_OBJ, HYP_CLS = 0.05, 1.0, 0.05
EPS = 1e-7
P = 128  # SBUF partitions
PAD_NEG = -40.0  # softplus(-40) ~ 4e-18: grid-plane padding value
NCONST = 12

_cache: dict = {}


def _build_program(layer_shapes, T, ocols):
    """Build the SPMD Bass program.

    layer_shapes: [(gh, gw)] * 3; T: padded slot-columns per layer
    (uniform); ocols: per-layer objectness-plane columns.
    Returns (nc, meta) with the accumulator column map.
    """
    import concourse.bass as bass
    import concourse.mybir as mybir
    import concourse.tile as tile

    f32 = mybir.dt.float32
    bf16 = mybir.dt.bfloat16
    ALU = mybir.AluOpType
    ACTF = mybir.ActivationFunctionType
    X = mybir.AxisListType.X
    COLS = NL * T
    OBJC = sum(ocols)
    o_offs = np.concatenate([[0], np.cumsum(ocols)]).astype(int)

    nc = bass.Bass()

    # three inputs, split so the chain-gating one is tiny:
    #   pc  (f32):  [posbox COLS*5 | ohvals COLS | consts NCONST*COLS]
    #   pcls(bf16): positive-row class logits [P, COLS*NCLS]
    #   obj (bf16): packed per-core objectness plane [P, OBJC]
    # bf16 halves the softplus-input DMA bytes and doubles the DVE reduce
    # rate; the softplus sums tolerate it (random +-0.2% element error).
    # consts layout per partition: 4 paired blocks [COLS,2] (awh2, b2min,
    # b2max, cxy2) then 4 single blocks [COLS] (w2h2pe, atan2c, wbox, wdedup)
    PCW = COLS * 6 + NCONST * COLS
    PC = nc.declare_dram_parameter("pc", [P, PCW], f32, isOutput=False)
    PCLS = nc.declare_dram_parameter("pcls", [P, COLS * NCLS], bf16, isOutput=False)
    OBJ = nc.declare_dram_parameter("objplane", [P, OBJC], bf16, isOutput=False)

    # accumulator column map
    col_box, col_corr, col_oh, col_cls = 0, 3, 6, 9
    col_grid = 12
    OUTC = 15
    OUT = nc.declare_dram_parameter("partial", [P, OUTC], f32, isOutput=True)

    with tile.TileContext(nc) as tc:
        with tc.tile_pool(name="small", bufs=1) as sm:
            # ---------- input loads ----------
            pc = sm.tile([P, PCW], f32)
            nc.sync.dma_start(out=pc[:], in_=PC[:])
            pcls = sm.tile([P, COLS * NCLS], bf16)
            nc.sync.dma_start(out=pcls[:], in_=PCLS[:])
            obj = sm.tile([P, OBJC], bf16)
            nc.sync.dma_start(out=obj[:], in_=OBJ[:])
            acc = sm.tile([P, OUTC], f32)

            pos_v = pc[:, : COLS * 5].rearrange("p (t c) -> p t c", c=5)
            ohg = pc[:, COLS * 5 : COLS * 6]
            _cb = COLS * 6  # consts base inside pc

            def paired(o):  # -> [P, COLS, 2]
                return pc[:, _cb + o * 2 * COLS : _cb + (o + 1) * 2 * COLS].rearrange(
                    "p (t c) -> p t c", c=2
                )

            def single(o):  # -> [P, COLS]
                s = _cb + 8 * COLS + o * COLS
                return pc[:, s : s + COLS]

            awh2, b2min, b2max, cxy2 = paired(0), paired(1), paired(2), paired(3)
            w2h2pe, atan2c, wbox, wdedup = single(0), single(1), single(2), single(3)

            # ---------- ACT phase 1: sigmoid + arctan (one table set) ------
            sig = sm.tile([P, COLS * 4], f32)
            sig_v = sig[:].rearrange("p (t c) -> p t c", c=4)
            nc.scalar.activation(out=sig_v[:], in_=pos_v[:, :, 0:4], func=ACTF.Sigmoid)

            _tn = [0]

            def pair_tile():
                _tn[0] += 1
                return sm.tile([P, COLS * 2], f32, name=f"pair{_tn[0]}")

            def pv(t):  # view [P, COLS, 2]
                return t[:].rearrange("p (t c) -> p t c", c=2)

            def stile():
                _tn[0] += 1
                return sm.tile([P, COLS], f32, name=f"s{_tn[0]}")

            V = nc.vector
            pxy, swh2, pwhh = pair_tile(), pair_tile(), pair_tile()
            b1min, b1max = pair_tile(), pair_tile()
            tmpa, tmpb = pair_tile(), pair_tile()

            V.tensor_scalar(pv(pxy)[:], sig_v[:, :, 0:2], 2.0, -0.5, ALU.mult, ALU.add)
            V.tensor_tensor(pv(swh2)[:], sig_v[:, :, 2:4], sig_v[:, :, 2:4], ALU.mult)
            V.tensor_tensor(pv(pwhh)[:], pv(swh2)[:], awh2[:], ALU.mult)

            # v-term argument FIRST so the ACT arctan (and with it the
            # exp/ln table switch) unblocks as early as possible:
            # w1/(h1+eps) == pwhh_x/(pwhh_y+eps/2)
            denh, q = stile(), stile()
            V.tensor_scalar(denh[:], pv(pwhh)[:, :, 1], EPS * 0.5, None, ALU.add)
            V.reciprocal(denh[:], denh[:])
            V.tensor_tensor(q[:], pv(pwhh)[:, :, 0], denh[:], ALU.mult)
            # ACT Arctan only supports [-pi/2, pi/2]; q > 0, so use
            # atan(q) = pi/2 - atan(1/q) for q > 1 (branchless select).
            rq, qm, at, mgt, u2 = stile(), stile(), stile(), stile(), stile()
            V.reciprocal(rq[:], q[:])
            V.tensor_tensor(qm[:], q[:], rq[:], ALU.min)
            i_at = nc.scalar.activation(out=at[:], in_=qm[:], func=ACTF.Arctan)

            V.tensor_tensor(pv(b1min)[:], pv(pxy)[:], pv(pwhh)[:], ALU.subtract)
            V.tensor_tensor(pv(b1max)[:], pv(pxy)[:], pv(pwhh)[:], ALU.add)

            # intersection
            V.tensor_tensor(pv(tmpa)[:], pv(b1max)[:], b2max[:], ALU.min)
            V.tensor_tensor(pv(tmpb)[:], pv(b1min)[:], b2min[:], ALU.max)
            V.tensor_tensor(pv(tmpa)[:], pv(tmpa)[:], pv(tmpb)[:], ALU.subtract)
            V.tensor_scalar(pv(tmpa)[:], pv(tmpa)[:], 0.0, None, ALU.max)  # relu
            inter = stile()
            V.tensor_tensor(inter[:], pv(tmpa)[:, :, 0], pv(tmpa)[:, :, 1], ALU.mult)
            # union (w1h1 = 4 * pwhh_x * pwhh_y; w2h2pe has +eps folded in)
            u, ru, iou = stile(), stile(), stile()
            V.tensor_tensor(u[:], pv(pwhh)[:, :, 0], pv(pwhh)[:, :, 1], ALU.mult)
            V.tensor_scalar(u[:], u[:], 4.0, None, ALU.mult)
            V.tensor_tensor(u[:], u[:], w2h2pe[:], ALU.add)
            V.tensor_tensor(u[:], u[:], inter[:], ALU.subtract)
            V.reciprocal(ru[:], u[:])
            V.tensor_tensor(iou[:], inter[:], ru[:], ALU.mult)
            # enclosing box diag^2 (c2 > 0 strictly since pwhh > 0: eps drop)
            V.tensor_tensor(pv(tmpa)[:], pv(b1max)[:], b2max[:], ALU.max)
            V.tensor_tensor(pv(tmpb)[:], pv(b1min)[:], b2min[:], ALU.min)
            V.tensor_tensor(pv(tmpa)[:], pv(tmpa)[:], pv(tmpb)[:], ALU.subtract)
            V.tensor_tensor(pv(tmpa)[:], pv(tmpa)[:], pv(tmpa)[:], ALU.mult)
            c2, rc2 = stile(), stile()
            V.tensor_tensor(c2[:], pv(tmpa)[:, :, 0], pv(tmpa)[:, :, 1], ALU.add)
            V.reciprocal(rc2[:], c2[:])
            # center distance^2
            V.tensor_tensor(pv(tmpb)[:], pv(pxy)[:], cxy2[:], ALU.subtract)
            V.tensor_tensor(pv(tmpb)[:], pv(tmpb)[:], pv(tmpb)[:], ALU.mult)
            rho2, rr = stile(), stile()
            V.tensor_tensor(rho2[:], pv(tmpb)[:, :, 0], pv(tmpb)[:, :, 1], ALU.add)
            V.tensor_tensor(rr[:], rho2[:], rc2[:], ALU.mult)
            # arctan range correction + v term
            V.tensor_scalar(mgt[:], q[:], 1.0, None, ALU.is_gt)
            V.tensor_scalar(u2[:], at[:], -2.0, float(np.pi / 2), ALU.mult, ALU.add)
            V.tensor_tensor(u2[:], mgt[:], u2[:], ALU.mult)
            V.tensor_tensor(at[:], at[:], u2[:], ALU.add)
            dat, v4 = stile(), stile()
            V.tensor_tensor(dat[:], atan2c[:], at[:], ALU.subtract)
            V.tensor_tensor(v4[:], dat[:], dat[:], ALU.mult)
            V.tensor_scalar(v4[:], v4[:], float(4.0 / np.pi**2), None, ALU.mult)
            ad, rad, alpha, va = stile(), stile(), stile(), stile()
            V.tensor_scalar(ad[:], iou[:], -1.0, 1.0 + EPS, ALU.mult, ALU.add)
            V.tensor_tensor(ad[:], ad[:], v4[:], ALU.add)
            V.reciprocal(rad[:], ad[:])
            V.tensor_tensor(alpha[:], v4[:], rad[:], ALU.mult)
            V.tensor_tensor(va[:], v4[:], alpha[:], ALU.mult)
            ciou = stile()
            V.tensor_tensor(ciou[:], iou[:], rr[:], ALU.subtract)
            V.tensor_tensor(ciou[:], ciou[:], va[:], ALU.subtract)

            # per-layer reductions from the ciou tile
            omc, rel, rp4 = stile(), stile(), stile()
            V.tensor_scalar(omc[:], ciou[:], -1.0, 1.0, ALU.mult, ALU.add)
            V.tensor_scalar(rel[:], ciou[:], 0.0, None, ALU.max)
            V.tensor_tensor(rp4[:], rel[:], pos_v[:, :, 4], ALU.mult)

            def lt(view):  # [P, COLS] -> [P, NL, T]
                return view.rearrange("p (l t) -> p l t", l=NL)

            boxm, corrm = stile(), stile()
            V.tensor_tensor(boxm[:], omc[:], wbox[:], ALU.mult)
            V.reduce_sum(acc[:, col_box : col_box + NL], lt(boxm[:]), X)
            V.tensor_tensor(corrm[:], rp4[:], wdedup[:], ALU.mult)
            V.reduce_sum(acc[:, col_corr : col_corr + NL], lt(corrm[:]), X)
            # ohvals padding is zero: no mask needed
            V.reduce_sum(acc[:, col_oh : col_oh + NL], lt(ohg), X)

            # ---------- ACT phase 2: softplus = ln(1 + exp(x)) -------------
            # (this compiler's table set lacks softplus; exp and ln share
            # natural_log_exp_and_others, so phase 2 costs one table load.)
            # ACT order: obj exp+ln first (its DVE reduces overlap the cls
            # exp/ln), then cls exp -> cls ln -> bf16 DVE reduce.
            sp_acts = []
            spge = sm.tile([P, OBJC], bf16)
            spgl = sm.tile([P, OBJC], bf16)
            i1 = nc.scalar.activation(out=spge[:], in_=obj[:], func=ACTF.Exp)
            i2 = nc.scalar.activation(out=spgl[:], in_=spge[:], func=ACTF.Ln, bias=1.0)
            sp_acts += [i1, i2]
            for l in range(NL):
                o0, o1 = int(o_offs[l]), int(o_offs[l + 1])
                V.reduce_sum(
                    acc[:, col_grid + l : col_grid + l + 1], spgl[:, o0:o1], X
                )

            # class logits: one exp over [P, COLS, 80], one ln, DVE reduce
            # (padding slots are zero -> ln 2 each; host subtracts them)
            spe = sm.tile([P, COLS * NCLS], f32)
            spc = sm.tile([P, COLS * NCLS], bf16)
            i1 = nc.scalar.activation(
                out=spe[:].rearrange("p (t c) -> p t c", c=NCLS),
                in_=pcls[:].rearrange("p (t c) -> p t c", c=NCLS),
                func=ACTF.Exp,
            )
            i2 = nc.scalar.activation(out=spc[:], in_=spe[:], func=ACTF.Ln, bias=1.0)
            sp_acts += [i1, i2]
            V.reduce_sum(
                acc[:, col_cls : col_cls + NL],
                spc[:].rearrange("p (l t) -> p l t", l=NL),
                X,
            )

            # enforce ACT ordering: all softplus after arctan (2 table loads
            # total: sigmoid_and_others once, ln/exp_and_others once)
            for i in sp_acts:
                tile.add_dep_helper(i.ins, i_at.ins, False, "act-table-grouping")

            # ---------- store partials (wait-cap hoists the extra sems) ----
            nc.sync.dma_start(out=OUT[:], in_=acc[:])

    _cap_sync_waits(nc, mybir)
    nc.finalize()
    meta = dict(
        COLS=COLS,
        T=T,
        OUTC=OUTC,
        ocols=ocols,
        col_box=col_box,
        col_corr=col_corr,
        col_oh=col_oh,
        col_cls=col_cls,
        col_grid=col_grid,
    )
    return nc, meta


def _cap_sync_waits(nc, mybir, maxw=1):
    """Compute-engine ISA encodings carry very few sync waits; Tile's
    scheduler can emit more (one per DMA sem lane).  Three rewrites, all
    semantics-preserving:
      1. drop waits on the instruction's own engine-completion semaphore
         (engine program order already guarantees them);
      2. hoist waits beyond `maxw` onto standalone EventSemaphore
         instructions placed just before the offender on the same engine;
      3. expand epilogue RANGE_CLEAR (this walrus build can't codegen it)
         into per-semaphore resets, but ONLY for semaphores the program
         actually touches - the full 0..255 sweep costs ~10.5 us of
         serial EventSemaphore instructions at ~140 ns each.
    """
    eng_sem = {
        "DVE": "DVE",
        "Activation": "Activation",
        "SP": "SP",
        "Pool": "Pool",
        "PE": "PE",
    }
    rc_opcode = 176  # NEURON_ISA_TPB_OPCODE_EVENT_SEMAPHORE_RANGE_CLEAR

    # pass 1: collect sem names and the set of sems the program touches
    sem_names = {}
    used = set()
    for bb in nc.m.functions[0].blocks:
        for inst in bb.instructions:
            if (
                type(inst).__name__ == "InstISA"
                and getattr(inst, "isa_opcode", None) == rc_opcode
            ):
                continue
            si = getattr(inst, "sync_info", None)
            if not si:
                continue
            for w in si.on_wait or []:
                sem_names[w.id] = w.ant_name
                used.add(w.id)
            for u in si.on_update or []:
                sem_names[u.id] = u.ant_name
                used.add(u.id)

    n = 0
    for bb in nc.m.functions[0].blocks:
        out = []
        for inst in bb.instructions:
            tname = type(inst).__name__
            if tname == "InstISA" and getattr(inst, "isa_opcode", None) == rc_opcode:
                start, end = inst.instr[13], inst.instr[14]
                for sid in range(start, end + 1):
                    if sid not in used:
                        continue
                    out.append(
                        mybir.InstEventSemaphore(
                            name=f"W-semreset-{sid}",
                            engine=inst.engine,
                            sync_info=mybir.SyncInfo(
                                on_wait=[],
                                on_update=[
                                    mybir.SyncUpdate(
                                        sync_type="semaphore",
                                        id=sid,
                                        update_mode="sem-wr-imm",
                                        update_value=0,
                                        ant_name=sem_names.get(sid, f"sem{sid}"),
                                    )
                                ],
                            ),
                        )
                    )
                continue
            si = getattr(inst, "sync_info", None)
            ow = list(si.on_wait) if (si and si.on_wait) else []
            if ow and tname != "InstEventSemaphore":
                epfx = eng_sem.get(str(inst.engine).split(".")[-1])
                if epfx:
                    keep0 = [
                        w for w in ow if not (w.ant_name or "").startswith(epfx + "_")
                    ]
                else:
                    keep0 = ow
                if len(keep0) > maxw:
                    excess, keep = keep0[:-maxw], keep0[-maxw:]
                    for w in excess:
                        n += 1
                        out.append(
                            mybir.InstEventSemaphore(
                                name=f"W-cap-{n}",
                                engine=inst.engine,
                                sync_info=mybir.SyncInfo(on_wait=[w], on_update=[]),
                            )
                        )
                else:
                    keep = keep0
                if len(keep) != len(ow):
                    si.on_wait = keep
            out.append(inst)
        bb.instructions = out


def _host_prep(inputs, T, ocols, meta):
    """Build per-core in_maps (numpy only)."""
    COLS = meta["COLS"]
    ps = [np.asarray(inputs[f"p{l}"]) for l in range(NL)]
    layer_shapes = [(p.shape[2], p.shape[3]) for p in ps]

    import ml_dtypes

    bf16 = ml_dtypes.bfloat16
    OBJC = sum(ocols)
    in_maps = [dict() for _ in range(NCORES)]
    posbox_k, poscls_k, consts_k, ohvals_k = [], [], [], []
    for k in range(NCORES):
        posbox_k.append(np.zeros((P, COLS * 5), np.float32))
        poscls_k.append(np.zeros((P, COLS * NCLS), bf16))
        ohvals_k.append(np.zeros((P, COLS), np.float32))
        cst = np.zeros((P, NCONST * COLS), np.float32)
        # benign defaults so padding slots stay finite through the CIoU math
        cst[:, 0 : 2 * COLS] = 1.0  # awh2
        cst[:, 8 * COLS : 9 * COLS] = 1.0  # w2h2pe
        consts_k.append(cst)
        # packed objectness plane, padded with PAD_NEG
        pm = np.empty((P, OBJC), bf16)
        in_maps[k]["objplane"] = pm
        o0 = 0
        for l in range(NL):
            ch4 = np.ascontiguousarray(
                ps[l][k * BPC : (k + 1) * BPC, :, :, :, 4], np.float32
            ).reshape(-1)
            cells = ch4.shape[0]
            buf = np.full(P * ocols[l], PAD_NEG, np.float32)
            buf[:cells] = ch4
            pm[:, o0 : o0 + ocols[l]] = buf.reshape(P, ocols[l]).astype(bf16)
            o0 += ocols[l]

    n_l = []
    pad_slots = 0  # padded positive slots across layers/cores (for lcls)
    for l in range(NL):
        gh, gw = layer_shapes[l]
        flat = ps[l].reshape(-1, NO)  # view, no copy
        rows_per_img = NA * gh * gw
        b = np.asarray(inputs[f"b{l}"]).astype(np.int64)
        a = np.asarray(inputs[f"a{l}"]).astype(np.int64)
        gj = np.asarray(inputs[f"gj{l}"]).astype(np.int64)
        gi = np.asarray(inputs[f"gi{l}"]).astype(np.int64)
        tc = np.asarray(inputs[f"tcls{l}"]).astype(np.int64)
        tb = np.asarray(inputs[f"tbox{l}"], np.float32)
        an = np.asarray(inputs[f"anch{l}"], np.float32)
        n = b.shape[0]
        n_l.append(n)
        # last-occurrence mask over global cells (images disjoint across cores)
        cell = ((b * NA + a) * gh + gj) * gw + gi
        seen = {}
        for r in range(n):
            seen[int(cell[r])] = r
        last = np.zeros(n, bool)
        last[list(seen.values())] = True

        c0 = l * T
        for k in range(NCORES):
            idxs = np.nonzero((b // BPC) == k)[0]
            cnt = idxs.shape[0]
            assert cnt <= P * T, f"layer {l} core {k}: {cnt} > {P * T}"
            pad_slots += P * T - cnt
            row = b[idxs] * rows_per_img + (
                (a[idxs] * gh + gj[idxs]) * gw + gi[idxs]
            )
            s = np.arange(cnt)
            pp, tcol = s % P, c0 + s // P
            rows = flat[row]
            posbox_k[k].reshape(P, COLS, 5)[pp, tcol] = rows[:, 0:5]
            poscls_k[k].reshape(P, COLS, NCLS)[pp, tcol] = rows[:, 5:NO].astype(bf16)
            ohvals_k[k][pp, tcol] = flat[row, 5 + (tc[idxs] - 1)]

            def setp(o, cx, cy):
                blk = consts_k[k][:, o * 2 * COLS : (o + 1) * 2 * COLS].reshape(
                    P, COLS, 2
                )
                blk[pp, tcol, 0] = cx
                blk[pp, tcol, 1] = cy

            def sets(o, val):
                blk = consts_k[k][:, 8 * COLS + o * COLS : 8 * COLS + (o + 1) * COLS]
                blk[pp, tcol] = val

            x2, y2, w2, h2 = tb[idxs, 0], tb[idxs, 1], tb[idxs, 2], tb[idxs, 3]
            setp(0, 2.0 * an[idxs, 0], 2.0 * an[idxs, 1])
            setp(1, x2 - w2 * 0.5, y2 - h2 * 0.5)
            setp(2, x2 + w2 * 0.5, y2 + h2 * 0.5)
            setp(3, x2, y2)
            sets(0, w2 * h2 + np.float32(EPS))
            sets(1, np.arctan(w2 / (h2 + np.float32(EPS))))
            sets(2, 1.0)
            sets(3, last[idxs].astype(np.float32))
    for k in range(NCORES):
        in_maps[k]["pc"] = np.hstack([posbox_k[k], ohvals_k[k], consts_k[k]])
        in_maps[k]["pcls"] = poscls_k[k]
    return in_maps, n_l, pad_slots, layer_shapes


def _combine(outs, n_l, pad_slots, layer_shapes, meta):
    """Host-side reduction of the 8 per-core [P, OUTC] partial tiles."""
    tot = np.zeros(meta["OUTC"], np.float64)
    for o in outs:
        tot += o.astype(np.float64).sum(axis=0)
    lbox = lobj = lcls = 0.0
    # padded positive slots contribute softplus(0) = ln 2 per class logit
    for l in range(NL):
        gh, gw = layer_shapes[l]
        G = B * NA * gh * gw
        box = tot[meta["col_box"] + l]
        corr = tot[meta["col_corr"] + l]
        oh = tot[meta["col_oh"] + l]
        clssp = tot[meta["col_cls"] + l] - meta["pad_l"][l] * NCLS * np.log(2.0)
        grid = tot[meta["col_grid"] + l]
        lbox += box / n_l[l]
        lobj += BALANCE[l] * (grid - corr) / G
        lcls += (clssp - oh) / (n_l[l] * NCLS)
    loss = (HYP_BOX * lbox + HYP_OBJ * lobj + HYP_CLS * lcls) * B
    return np.float32(loss)


def _get_program(inputs):
    ps = [np.asarray(inputs[f"p{l}"]) for l in range(NL)]
    layer_shapes = [(p.shape[2], p.shape[3]) for p in ps]
    # padded slot columns (uniform across layers) from worst-case per-core
    T = 1
    for l in range(NL):
        b = np.asarray(inputs[f"b{l}"]).astype(np.int64)
        mx = max(int(((b // BPC) == k).sum()) for k in range(NCORES))
        T = max(T, -(-mx // P))
    ocols = tuple(
        -(-(BPC * NA * gh * gw) // P) for gh, gw in layer_shapes
    )
    key = (tuple(layer_shapes), T, ocols)
    if key not in _cache:
        _cache[key] = _build_program(layer_shapes, T, ocols)
    return _cache[key], T, ocols


last_result = None  # BassKernelResults of the most recent run (for profiling)


def kernel(**inputs) -> np.ndarray:
    global last_result
    (nc, meta), T, ocols = _get_program(inputs)
    in_maps, n_l, pad_slots, layer_shapes = _host_prep(inputs, T, ocols, meta)
    # per-layer padded-slot counts for the lcls correction
    meta["pad_l"] = [P * T * NCORES - n_l[l] for l in range(NL)]
    from concourse.bass_utils import run_bass_kernel_spmd

    trace = bool(int(os.environ.get("DETLOSS_TRACE", "0")))
    if trace:
        # NTFF profiling needs an initialized PJRT client in this
        # interpreter; warm up with an untraced run first.
        run_bass_kernel_spmd(nc, in_maps, list(range(NCORES)))
    res = run_bass_kernel_spmd(nc, in_maps, list(range(NCORES)), trace=trace)
    last_result = res
    outs = [res.results[k]["partial"] for k in range(NCORES)]
    return _combine(outs, n_l, pad_slots, layer_shapes, meta)
